# revision 23
# baseline (speedup 1.0000x reference)
"""HBV hydrology model (nn_HBVMul) Trainium2 Bass kernel — v2.

Sharding: data-parallel over the 1500-grid axis across 8 cores (192 grids/core,
padded to 1536). Per-core lane layout: partition p = g_lo*16 + mu (g_lo in 0..7,
mu in 0..15), free dim g_hi in 0..23; local grid = g_lo*24 + g_hi.

v2 restructure (validated in numpy against the jax reference and on HW):
  - Exogenous per-step series (snowfall s, melt/refreeze budget ab, rain R,
    evap factors F = min(E/(LP*FC), 1) and Fc = 1-F) are computed host-side
    and DMA'd in; they are pure elementwise transforms of the inputs.
  - Activation-table pin: Ln/Exp forced into the shared
    natural_log_exp_and_others table (else the placement pass reloads the
    act table on every Ln<->Exp switch, ~1.3us twice per step).
  - Software-pipelined step cycle (~26 ops): cycle t runs the soil tail of
    step t-1 (rech..d2 via d2 = max(SMc*Fc, SMc-E), ET = min(SMc*F, E)
    exactly off the critical loop), the soil head of step t (m0 -> Act ln ->
    w1 -> Act exp), snow for step t, and the SUZ chain for step t-2 —
    ordered so every RAW dependency has >=1 unrelated op between producer
    and consumer (dependent back-to-back DVE ops cost ~210ns vs ~122ns) and
    both Act round trips have ~7 filler slots.
  - Inputs split into halves: h1 of chunk ci+1 is DMA-prefetched mid-body
    through +TC-shifted dram views (tensors padded by TC rows); the post
    pass (Q1/Q2/Qs on Pool, folded SLZ scan z' = (PERC+z)*(1-k2) on DVE,
    mu-mean matmuls on PE, staging copies on Act) runs in two halves so it
    overlaps the step loop.
  - Pool HW ISA only supports TT add/sub/mult (+ TensorScalar): no
    TensorScalarPtr/min/max there, so the state chains all stay on DVE.
"""

import os
import sys
import numpy as np

sys.path.insert(0, "/opt/trn_rl_repo")

NSTEP, NGRID, MU, LENF = 730, 1500, 16, 15
PRECS = 1e-5
NC_CORES = 8
G = 192          # grids per core
GL, GH = 8, 24   # g_lo x g_hi split of the 192 grids
P = 128          # partitions = GL * MU
NCH, TC = 10, 73  # time chunks
TSUB = [(0, 19), (19, 18), (37, 18), (55, 18)]  # mu-mean matmul sub-slices

PARA_SCALE = np.array([[1, 6], [50, 1000], [0.05, 0.9], [0.01, 0.5], [0.001, 0.2],
                       [0.2, 1], [0, 10], [0, 100], [-2.5, 2.5], [0.5, 10],
                       [0, 0.1], [0, 0.2]], dtype=np.float32)
ROUT_SCALE = np.array([[0, 2.9], [0, 6.5]], dtype=np.float32)

_PROGRAM_CACHE = {}


def _build_program():
    import concourse.bacc as bacc
    import concourse.bass as bass
    import concourse.tile as tile
    import concourse.mybir as mybir
    from concourse.bass import ts

    dt = mybir.dt
    Alu = mybir.AluOpType
    Act = mybir.ActivationFunctionType

    # Pin Ln/Exp to the combined activation table (json index 6,
    # natural_log_exp_and_others). The table-load placement pass picks, per
    # activation, some table containing its function; with Ln in
    # natural_log (5) and Exp in exp_and_others (0) it reloads the table on
    # every Ln<->Exp switch (~1.3us each, twice per step). Hiding Ln/Exp
    # from all other tables (ids/order unchanged, so the emitted
    # act_func_set_id stays valid) forces the shared table and hoists the
    # single load out of the loop.
    from concourse.hw_specs import get_activation_tables as _gat

    def _gat_pinned(arch):
        tabs = _gat(arch)
        keep = "natural_log_exp_and_others"
        drop = {mybir.ActivationFunctionType.Ln, mybir.ActivationFunctionType.Exp}
        return {name: (funcs if name == keep else funcs - drop)
                for name, funcs in tabs.items()}

    bacc.get_activation_tables = _gat_pinned

    nc = bacc.Bacc("TRN2", target_bir_lowering=False, debug=False,
                   num_devices=NC_CORES)

    f32 = dt.float32
    # padded by TC rows so the mid-chunk prefetch of chunk ci+1 stays in range
    NPADT = NSTEP + TC
    ab_ap = nc.dram_tensor("ab48", [NPADT, P, 48], f32, kind="ExternalInput").ap()
    rb_ap = nc.dram_tensor("rb", [NPADT, P, GH], f32, kind="ExternalInput").ap()
    fcb_ap = nc.dram_tensor("fcb", [NPADT, P, GH], f32, kind="ExternalInput").ap()
    fb_ap = nc.dram_tensor("fb", [NPADT, P, GH], f32, kind="ExternalInput").ap()
    eb_ap = nc.dram_tensor("eb", [NPADT, P, GH], f32, kind="ExternalInput").ap()
    par_ap = nc.dram_tensor("par", [P, 12, GH], f32, kind="ExternalInput").ap()
    rt_ap = nc.dram_tensor("rt", [G, 2], f32, kind="ExternalInput").ap()
    wm_ap = nc.dram_tensor("wmean", [P, GL], f32, kind="ExternalInput").ap()
    id_ap = nc.dram_tensor("id128", [P, P], f32, kind="ExternalInput").ap()
    lntk_ap = nc.dram_tensor("lntk", [1, LENF], f32, kind="ExternalInput").ap()
    tk_ap = nc.dram_tensor("tk", [1, LENF], f32, kind="ExternalInput").ap()
    out_ap = nc.dram_tensor("out", [NSTEP, G, 5], f32, kind="ExternalOutput").ap()

    scr = {}
    for name in ["smqs", "smq0", "smq1", "smq2", "smet"]:
        scr[name] = nc.dram_tensor(name, [NSTEP, G], f32, kind="Internal").ap()

    with tile.TileContext(nc) as tc:
        from contextlib import ExitStack
        ctx = ExitStack()
        with ctx:
            consts = ctx.enter_context(tc.tile_pool(name="consts", bufs=1))
            chunk = ctx.enter_context(tc.tile_pool(name="chunk", bufs=1))
            step = ctx.enter_context(tc.tile_pool(name="step", bufs=4))
            post = ctx.enter_context(tc.tile_pool(name="post", bufs=2))
            psum = ctx.enter_context(tc.tile_pool(name="psum", bufs=2, space="PSUM"))

            V = nc.vector
            S = nc.scalar
            GP = nc.gpsimd

            # ---- Phase 0: parameters ----
            par_sb = consts.tile([P, 12, GH], f32)
            nc.sync.dma_start(out=par_sb[:], in_=par_ap)
            pp_ = {}
            # param indices: 0 BETA, 1 FC, 2 K0, 3 K1, 4 K2, 5 LP, 6 PERC,
            #                7 UZL, 8 TT, 9 CFMAX, 10 CFR, 11 CWH
            for j in (0, 1, 2, 3, 4, 6, 7, 11):
                pt = consts.tile([P, GH], f32, tag=f"par{j}", name=f"par{j}")
                lo, hi = float(PARA_SCALE[j, 0]), float(PARA_SCALE[j, 1])
                V.tensor_scalar(pt[:], par_sb[:, j, :], hi - lo, lo,
                                Alu.mult, Alu.add)
                pp_[j] = pt
            betab, FCb, k0b, k1b, k2b = pp_[0], pp_[1], pp_[2], pp_[3], pp_[4]
            ppb, uzlb, CWHb = pp_[6], pp_[7], pp_[11]

            def ctile(tag, w=GH):
                return consts.tile([P, w], f32, tag=tag, name=tag)

            kcb = ctile("kcb");     V.tensor_scalar(kcb[:], CWHb[:], 1.0, None, Alu.add)
            iFCb = ctile("ifc");    V.reciprocal(iFCb[:], FCb[:])
            omk1b = ctile("omk1");  V.tensor_scalar(omk1b[:], k1b[:], -1.0, 1.0, Alu.mult, Alu.add)
            aslzb = ctile("aslz");  V.tensor_scalar(aslzb[:], k2b[:], -1.0, 1.0, Alu.mult, Alu.add)
            ralz = ctile("ralz");   V.reciprocal(ralz[:], aslzb[:])
            kqb = ctile("kqb");     V.tensor_tensor(kqb[:], k2b[:], ralz[:], Alu.mult)

            wm_sb = consts.tile([P, GL], f32)
            nc.sync.dma_start(out=wm_sb[:], in_=wm_ap)
            id_sb = consts.tile([P, P], f32)
            nc.sync.dma_start(out=id_sb[:], in_=id_ap)

            # ---- states ----
            SW48 = consts.tile([P, 48], f32, tag="SW48", name="SW48")
            V.memset(SW48[:, 0:GH], 0.001)   # SP
            V.memset(SW48[:, GH:48], 0.002)  # W = SNOWPACK + MELTWATER
            d2s = consts.tile([P, GH], f32, tag="d2s", name="d2s")
            V.memset(d2s[:], 0.001)          # raw SM (clamp applied at use)
            SUZs = consts.tile([P, GH], f32, tag="SUZ", name="SUZ")
            V.memset(SUZs[:], 0.001)
            SLZl = consts.tile([P, GH], f32, tag="SLZ", name="SLZ")
            V.memset(SLZl[:], 0.001)

            # ---- chunk buffers ----
            IH = 37   # input half split: h1=[0,IH) prefetched, h2=[IH,TC)
            OH = 34   # output half split for the mid-chunk post pass

            def cbuf(tag, n, w=GH):
                return chunk.tile([P, n, w], f32, tag=tag, name=tag)
            ABh = [cbuf("ABh1", IH, 48), cbuf("ABh2", TC - IH, 48)]
            Rh = [cbuf("Rh1", IH), cbuf("Rh2", TC - IH)]
            Fch = [cbuf("Fch1", IH), cbuf("Fch2", TC - IH)]
            Fh = [cbuf("Fh1", IH), cbuf("Fh2", TC - IH)]
            Eh = [cbuf("Eh1", IH), cbuf("Eh2", TC - IH)]

            def hv(bufs, t):  # half-split view of input series at step t
                return bufs[0][:, t, :] if t < IH else bufs[1][:, t - IH, :]

            ib = cbuf("ib", TC); rub = cbuf("rub", TC)
            PERCb = cbuf("PERCb", TC); Q0b = cbuf("Q0b", TC); yb = cbuf("yb", TC)
            ETb = cbuf("ETb", TC); Q1b = cbuf("Q1b", TC)
            zb = cbuf("zb", TC)
            Q2b = cbuf("Q2b", TC); Qsb = cbuf("Qsb", TC)

            def bcr(t, n):  # broadcast [P, GH] param over n steps
                return t[:, None, :].to_broadcast([P, n, GH])

            scr_views = {k: v.rearrange("(c t) (gl gh) -> c gl t gh", c=NCH, gl=GL)
                         for k, v in scr.items()}

            def stile(tag, w=GH):
                return step.tile([P, w], f32, tag=tag, name=tag)

            def emit_post(ci, h0, hn, subs):
                """Batched post for output positions [h0, h0+hn): Q1/q2t/Q2/Qs
                on Pool, SLZ scans on DVE, mu-means on PE."""
                sl = slice(h0, h0 + hn)
                GP.tensor_tensor(Q1b[:, sl, :], bcr(k1b, hn), yb[:, sl, :], Alu.mult)
                for g in range(GH):
                    # z' = (PERC[t] + z) * (1-k2): SLZ update as one scan
                    V.tensor_tensor_scan(
                        zb[:, sl, g], PERCb[:, sl, g],
                        aslzb[:, g:g + 1].to_broadcast([P, hn]),
                        SLZl[:, g:g + 1], Alu.add, Alu.mult)
                V.tensor_copy(out=SLZl[:], in_=zb[:, h0 + hn - 1, :])
                GP.tensor_tensor(Q2b[:, sl, :], zb[:, sl, :], bcr(kqb, hn), Alu.mult)
                GP.tensor_tensor(Qsb[:, sl, :], Q0b[:, sl, :], Q1b[:, sl, :], Alu.add)
                GP.tensor_tensor(Qsb[:, sl, :], Qsb[:, sl, :], Q2b[:, sl, :], Alu.add)
                for buf, name in ((Qsb, "smqs"), (Q0b, "smq0"), (Q1b, "smq1"),
                                  (Q2b, "smq2"), (ETb, "smet")):
                    for (t0, tl) in subs:
                        ps = psum.tile([GL, 20 * GH], f32, tag="msum", name="msum")
                        rhs = buf[:, t0:t0 + tl, :].rearrange("p t g -> p (t g)")
                        nc.tensor.matmul(ps[:, :tl * GH], wm_sb[:], rhs,
                                         start=True, stop=True)
                        stg = post.tile([GL, 20 * GH], f32, tag="mstg", name="mstg")
                        S.copy(stg[:, :tl * GH], ps[:, :tl * GH])
                        dst = scr_views[name][ci][:, t0:t0 + tl, :]
                        nc.sync.dma_start(
                            out=dst,
                            in_=stg[:, :tl * GH].rearrange("m (t g) -> m t g", g=GH))

            SUBS_H1 = [(0, 17), (17, 17)]
            SUBS_H2 = [(OH, 20), (OH + 20, TC - OH - 20)]
            in_srcs = ((ABh, ab_ap), (Rh, rb_ap), (Fch, fcb_ap),
                       (Fh, fb_ap), (Eh, eb_ap))
            # shifted base views for the mid-chunk prefetch of chunk ci+1
            in_shift = {id(bufs): src[TC:TC + NSTEP] for bufs, src in in_srcs}

            # preload chunk 0 h1 before the loop
            for bufs, src in in_srcs:
                sl = src[0:IH].rearrange("t p g -> p t g")
                nc.sync.dma_start(out=bufs[0][:], in_=sl)

            # Software-pipelined step cycle. Cycle t computes, interleaved so
            # every RAW dependency has >=1 unrelated op between producer and
            # consumer (dependent back-to-back DVE ops cost ~210ns vs ~122ns):
            #   soil tail of step t-1 (rech..d2/ET/ru, using sw issued last
            #   cycle, so the Act exp round-trip has a full cycle of slack),
            #   soil head of step t (m0 -> ln -> w1 -> exp),
            #   snow for step t, and the full SUZ chain for step t-2.
            with tc.For_i(0, NCH, 1) as ci:
                # current chunk h2 loads (h1 prefetched during the previous
                # iteration; chunk 0 h1 preloaded above)
                for bufs, src in in_srcs:
                    sl = src[ts(ci, TC)][IH:].rearrange("t p g -> p t g")
                    nc.sync.dma_start(out=bufs[1][:], in_=sl)

                def head_ops(t, UW):
                    """soil head + snow for step t; returns (UW_next, sw)."""
                    # emitted inline by emit_cycle; only used for t=0 prologue
                    m0 = stile("m0")
                    V.scalar_tensor_tensor(m0[:], d2s[:], PRECS, iFCb[:],
                                           Alu.max, Alu.mult)
                    S.activation(m0[:], m0[:], Act.Ln); l = m0
                    V.scalar_tensor_tensor(SW48[:, 0:GH], UW[:, 0:GH], 0.0,
                                           UW[:, GH:48], Alu.max, Alu.min)
                    v_ = stile("v_")
                    V.tensor_tensor(v_[:], kcb[:], SW48[:, 0:GH], Alu.mult)
                    V.tensor_tensor(SW48[:, GH:48], v_[:], UW[:, GH:48], Alu.min)
                    w1 = stile("w1"); V.tensor_tensor(w1[:], betab[:], l[:], Alu.mult)
                    S.activation(w1[:], w1[:], Act.Exp); sw = w1
                    q = stile("q")
                    V.tensor_tensor(q[:], UW[:, GH:48], SW48[:, GH:48], Alu.subtract)
                    V.tensor_tensor(ib[:, t, :], hv(Rh, t), q[:], Alu.add)
                    UW2 = stile("UW", 48)
                    V.tensor_tensor(UW2[:], SW48[:], hv(ABh, t + 1), Alu.add)
                    return UW2, sw

                # t=0 prologue
                gate_w = stile("gate_w", 1); V.memset(gate_w[:], 0.0)
                gate_r = stile("gate_r", 1); V.memset(gate_r[:], 0.0)
                UW = stile("UW", 48)
                V.tensor_tensor(UW[:], SW48[:], hv(ABh, 0), Alu.add)
                UW, sw_prev = head_ops(0, UW)

                def tail_ops(u, sw_u, zstep):
                    """soil tail for step u interleaved with the SUZ chain for
                    step zstep (None to skip); emits in dependency-spaced
                    order. Returns nothing; updates d2s/SUZs states."""
                    z = zstep is not None
                    rech = stile("rech")
                    V.tensor_tensor(rech[:], sw_u[:], ib[:, u, :], Alu.mult)
                    if z:
                        uu = stile("uu")
                        V.tensor_tensor(uu[:], SUZs[:], rub[:, zstep, :], Alu.add)
                    SMa = stile("SMa")
                    V.scalar_tensor_tensor(SMa[:], d2s[:], PRECS, ib[:, u, :],
                                           Alu.max, Alu.add)
                    if z:
                        V.tensor_tensor(PERCb[:, zstep, :], uu[:], ppb[:], Alu.min)
                    SMb = stile("SMb")
                    V.tensor_tensor(SMb[:], SMa[:], rech[:], Alu.subtract)
                    SMc = stile("SMc")
                    V.tensor_tensor(SMc[:], SMb[:], FCb[:], Alu.min)
                    if z:
                        vv = stile("vv")
                        V.tensor_tensor(vv[:], uu[:], PERCb[:, zstep, :], Alu.subtract)
                    a1 = stile("a1")
                    V.tensor_tensor(a1[:], SMc[:], hv(Fch, u), Alu.mult)
                    a2 = stile("a2")
                    V.tensor_tensor(a2[:], SMc[:], hv(Eh, u), Alu.subtract)
                    if z:
                        t1 = stile("t1")
                        V.tensor_tensor(t1[:], vv[:], uzlb[:], Alu.min)
                    V.tensor_tensor(d2s[:], a1[:], a2[:], Alu.max)
                    V.tensor_tensor(rub[:, u, :], SMa[:], SMc[:], Alu.subtract)
                    if z:
                        x0 = stile("x0")
                        V.tensor_tensor(x0[:], vv[:], t1[:], Alu.subtract)
                    zF = stile("zF")
                    V.tensor_tensor(zF[:], SMc[:], hv(Fh, u), Alu.mult)
                    V.tensor_tensor(ETb[:, u, :], zF[:], hv(Eh, u), Alu.min)
                    if z:
                        V.tensor_tensor(Q0b[:, zstep, :], k0b[:], x0[:], Alu.mult)
                        V.tensor_tensor(yb[:, zstep, :], vv[:], Q0b[:, zstep, :],
                                        Alu.subtract)
                        V.tensor_tensor(SUZs[:], omk1b[:], yb[:, zstep, :], Alu.mult)

                for t in range(1, TC):
                    u, v = t - 1, t - 2  # soil-tail step, suz step
                    z = v >= 0
                    # 26-slot cycle; SM loop (rech->SMb->SMc->a1/a2->d2->m0->
                    # ln->w1->exp->wrap rech) saturates it with 7-slot gaps
                    # around both Act round trips.
                    # s0 rech(u) = (sw + 0)*i; the zero gate tile is
                    # written late in the previous cycle so the scheduler
                    # cannot enqueue rech before the exp result is near
                    rech = stile("rech")
                    V.scalar_tensor_tensor(rech[:], sw_prev[:], gate_r[:, 0:1],
                                           ib[:, u, :], Alu.add, Alu.mult)
                    # s1 SMa(u)
                    SMa = stile("SMa")
                    V.scalar_tensor_tensor(SMa[:], d2s[:], PRECS, ib[:, u, :],
                                           Alu.max, Alu.add)
                    # s2 z1: uu = SUZ + ru[v]
                    if z:
                        uu = stile("uu")
                        V.tensor_tensor(uu[:], SUZs[:], rub[:, v, :], Alu.add)
                    # s3 SMb(u)
                    SMb = stile("SMb")
                    V.tensor_tensor(SMb[:], SMa[:], rech[:], Alu.subtract)
                    # s4 z2: PERC
                    if z:
                        V.tensor_tensor(PERCb[:, v, :], uu[:], ppb[:], Alu.min)
                    # s5 SMc(u)
                    SMc = stile("SMc")
                    V.tensor_tensor(SMc[:], SMb[:], FCb[:], Alu.min)
                    # s6 z3: vv
                    if z:
                        vv = stile("vv")
                        V.tensor_tensor(vv[:], uu[:], PERCb[:, v, :], Alu.subtract)
                    # s7 a1 = SMc*Fc[u]
                    a1 = stile("a1")
                    V.tensor_tensor(a1[:], SMc[:], hv(Fch, u), Alu.mult)
                    # s8 a2 = SMc - E[u]
                    a2 = stile("a2")
                    V.tensor_tensor(a2[:], SMc[:], hv(Eh, u), Alu.subtract)
                    # s9 z4: t1
                    if z:
                        t1 = stile("t1")
                        V.tensor_tensor(t1[:], vv[:], uzlb[:], Alu.min)
                    # s10 d2 = max(a1, a2)
                    V.tensor_tensor(d2s[:], a1[:], a2[:], Alu.max)
                    # s11 z5: x0
                    if z:
                        x0 = stile("x0")
                        V.tensor_tensor(x0[:], vv[:], t1[:], Alu.subtract)
                    # s12 m0(t) -> ln
                    m0 = stile("m0")
                    V.scalar_tensor_tensor(m0[:], d2s[:], PRECS, iFCb[:],
                                           Alu.max, Alu.mult)
                    S.activation(m0[:], m0[:], Act.Ln); l = m0
                    # s13 z6: Q0
                    if z:
                        V.tensor_tensor(Q0b[:, v, :], k0b[:], x0[:], Alu.mult)
                    # s14 n2: SP' (snow stays on DVE: Pool HW lacks
                    # TensorScalarPtr and min/max TensorTensor)
                    V.scalar_tensor_tensor(SW48[:, 0:GH], UW[:, 0:GH], 0.0,
                                           UW[:, GH:48], Alu.max, Alu.min)
                    # s15 z7: y
                    if z:
                        V.tensor_tensor(yb[:, v, :], vv[:], Q0b[:, v, :], Alu.subtract)
                    # s16 n3: v = kc*SP'
                    v_ = stile("v_")
                    V.tensor_tensor(v_[:], kcb[:], SW48[:, 0:GH], Alu.mult)
                    # s17 z8: SUZ'
                    if z:
                        V.tensor_tensor(SUZs[:], omk1b[:], yb[:, v, :], Alu.mult)
                    # s18 n4: W'
                    V.tensor_tensor(SW48[:, GH:48], v_[:], UW[:, GH:48], Alu.min)
                    gate_w = stile("gate_w", 1); V.memset(gate_w[:], 0.0)
                    # s19 w1 = (l + 0)*beta -> exp (gated like rech)
                    w1 = stile("w1")
                    V.scalar_tensor_tensor(w1[:], l[:], gate_w[:, 0:1], betab[:],
                                           Alu.add, Alu.mult)
                    S.activation(w1[:], w1[:], Act.Exp); sw = w1
                    # s20 n5: q
                    q = stile("q")
                    V.tensor_tensor(q[:], UW[:, GH:48], SW48[:, GH:48], Alu.subtract)
                    # s21 zF(u) = SMc * F[u]   (exact ET path, off the SM loop)
                    zF = stile("zF")
                    V.tensor_tensor(zF[:], SMc[:], hv(Fh, u), Alu.mult)
                    # s22 n6: i
                    V.tensor_tensor(ib[:, t, :], hv(Rh, t), q[:], Alu.add)
                    # s23 ET(u) = min(zF, E[u])
                    V.tensor_tensor(ETb[:, u, :], zF[:], hv(Eh, u), Alu.min)
                    # s24 n1+: UW(t+1)
                    if t < TC - 1:
                        UW = stile("UW", 48)
                        V.tensor_tensor(UW[:], SW48[:], hv(ABh, t + 1), Alu.add)
                    # s25 ru(u)
                    V.tensor_tensor(rub[:, u, :], SMa[:], SMc[:], Alu.subtract)
                    gate_r = stile("gate_r", 1); V.memset(gate_r[:], 0.0)
                    sw_prev = sw

                    if t == 36:
                        # prefetch next chunk's AB/R h1 (their h1 reads end at
                        # cycle 36); overlaps the remaining cycles
                        for bufs in (ABh, Rh):
                            sl = in_shift[id(bufs)][ts(ci, TC)][0:IH] \
                                .rearrange("t p g -> p t g")
                            nc.sync.dma_start(out=bufs[0][:], in_=sl)
                        # first-half post (outputs complete through OH-1)
                        emit_post(ci, 0, OH, SUBS_H1)
                    elif t == 37:
                        # Fc/E reads lag one cycle (index u=t-1): h1 clear now
                        for bufs in (Fch, Fh, Eh):
                            sl = in_shift[id(bufs)][ts(ci, TC)][0:IH] \
                                .rearrange("t p g -> p t g")
                            nc.sync.dma_start(out=bufs[0][:], in_=sl)

                # epilogue: soil tail for TC-1 with suz TC-2, then suz TC-1
                tail_ops(TC - 1, sw_prev, TC - 2)
                vv = stile("vv"); uu = stile("uu")
                s = TC - 1
                V.tensor_tensor(uu[:], SUZs[:], rub[:, s, :], Alu.add)
                V.tensor_tensor(PERCb[:, s, :], uu[:], ppb[:], Alu.min)
                V.tensor_tensor(vv[:], uu[:], PERCb[:, s, :], Alu.subtract)
                t1 = stile("t1"); V.tensor_tensor(t1[:], vv[:], uzlb[:], Alu.min)
                x0 = stile("x0"); V.tensor_tensor(x0[:], vv[:], t1[:], Alu.subtract)
                V.tensor_tensor(Q0b[:, s, :], k0b[:], x0[:], Alu.mult)
                V.tensor_tensor(yb[:, s, :], vv[:], Q0b[:, s, :], Alu.subtract)
                V.tensor_tensor(SUZs[:], omk1b[:], yb[:, s, :], Alu.mult)

                # second-half post
                emit_post(ci, OH, TC - OH, SUBS_H2)

            # ---- finale: routing conv + output assembly ----
            gparts = [(0, 128), (128, 64)]
            wtaps = []
            for (p0, pl) in gparts:
                rts = post.tile([pl, 2], f32, tag=f"rts{p0}", name=f"rts{p0}")
                nc.sync.dma_start(out=rts[:], in_=rt_ap[p0:p0 + pl, :])
                am1 = post.tile([pl, 1], f32, tag=f"am1{p0}", name=f"am1{p0}")
                V.tensor_scalar(am1[:], rts[:, 0:1], 2.9, 0.0, Alu.mult, Alu.max)
                V.tensor_scalar(am1[:], am1[:], -0.9, None, Alu.add)
                th = post.tile([pl, 1], f32, tag=f"th{p0}", name=f"th{p0}")
                V.tensor_scalar(th[:], rts[:, 1:2], 6.5, 0.0, Alu.mult, Alu.max)
                V.tensor_scalar(th[:], th[:], 0.5, None, Alu.add)
                ivt = post.tile([pl, 1], f32, tag=f"ivt{p0}", name=f"ivt{p0}")
                V.reciprocal(ivt[:], th[:])
                lnt = post.tile([pl, LENF], f32, tag=f"lnt{p0}", name=f"lnt{p0}")
                nc.sync.dma_start(out=lnt[:], in_=lntk_ap.to_broadcast([pl, LENF]))
                tkb = post.tile([pl, LENF], f32, tag=f"tkb{p0}", name=f"tkb{p0}")
                nc.sync.dma_start(out=tkb[:], in_=tk_ap.to_broadcast([pl, LENF]))
                e1 = post.tile([pl, LENF], f32, tag=f"e1{p0}", name=f"e1{p0}")
                V.tensor_scalar(e1[:], lnt[:], am1[:, 0:1], None, Alu.mult)
                e2 = post.tile([pl, LENF], f32, tag=f"e2{p0}", name=f"e2{p0}")
                V.tensor_scalar(e2[:], tkb[:], ivt[:, 0:1], None, Alu.mult)
                V.tensor_tensor(e1[:], e1[:], e2[:], Alu.subtract)
                vt = post.tile([pl, LENF], f32, tag=f"vt{p0}", name=f"vt{p0}")
                S.activation(vt[:], e1[:], Act.Exp)
                ssum = post.tile([pl, 1], f32, tag=f"ss{p0}", name=f"ss{p0}")
                V.tensor_reduce(ssum[:], vt[:], mybir.AxisListType.X, Alu.add)
                rs = post.tile([pl, 1], f32, tag=f"rs{p0}", name=f"rs{p0}")
                V.reciprocal(rs[:], ssum[:])
                wt = post.tile([pl, LENF], f32, tag=f"wt{p0}", name=f"wt{p0}")
                V.tensor_scalar(wt[:], vt[:], rs[:, 0:1], None, Alu.mult)
                wtaps.append(wt)

            tblocks = [(i * 128, min(128, NSTEP - i * 128)) for i in range(6)]

            # transposed-load Qsimave into [grid, time] padded tiles
            xps = []
            for (p0, pl) in gparts:
                xp = post.tile([pl, LENF - 1 + NSTEP], f32, tag=f"xp{p0}", name=f"xp{p0}")
                V.memset(xp[:], 0.0)
                xps.append(xp)
            for (t0, tl) in tblocks:
                ld = post.tile([128, G], f32, tag="qsld", name="qsld")
                nc.sync.dma_start(out=ld[:tl, :], in_=scr["smqs"][t0:t0 + tl, :])
                for xi, (p0, pl) in enumerate(gparts):
                    pst = psum.tile([pl, 128], f32, tag="ptr", name="ptr")
                    nc.tensor.transpose(pst[:, :tl], ld[:tl, p0:p0 + pl],
                                        id_sb[:tl, :tl])
                    V.tensor_copy(out=xps[xi][:, LENF - 1 + t0:LENF - 1 + t0 + tl],
                                  in_=pst[:, :tl])

            accs = []
            for xi, (p0, pl) in enumerate(gparts):
                acc = post.tile([pl, NSTEP], f32, tag=f"acc{p0}", name=f"acc{p0}")
                acc2 = post.tile([pl, NSTEP], f32, tag=f"acc2{p0}", name=f"acc2{p0}")
                xp, wt = xps[xi], wtaps[xi]
                V.tensor_scalar(acc[:], xp[:, LENF - 1:LENF - 1 + NSTEP],
                                wt[:, 0:1], None, Alu.mult)
                cur, nxt = acc, acc2
                for k in range(1, LENF):
                    V.scalar_tensor_tensor(
                        nxt[:], xp[:, LENF - 1 - k:LENF - 1 - k + NSTEP],
                        wt[:, k:k + 1], cur[:], Alu.mult, Alu.add)
                    cur, nxt = nxt, cur
                accs.append(cur)

            for (t0, tl) in tblocks:
                packs = post.tile([128, G, 5], f32, tag="packs", name="packs")
                for j, name in enumerate(["smq0", "smq1", "smq2", "smet"]):
                    ld = post.tile([128, G], f32, tag=f"mld{j}", name=f"mld{j}")
                    nc.sync.dma_start(out=ld[:tl, :], in_=scr[name][t0:t0 + tl, :])
                    V.tensor_copy(out=packs[:tl, :, j + 1], in_=ld[:tl, :])
                for xi, (p0, pl) in enumerate(gparts):
                    pst = psum.tile([128, 128], f32, tag="ptr2", name="ptr2")
                    nc.tensor.transpose(pst[:tl, :pl], accs[xi][:, t0:t0 + tl],
                                        id_sb[:pl, :pl])
                    V.tensor_copy(out=packs[:tl, p0:p0 + pl, 0], in_=pst[:tl, :pl])
                nc.sync.dma_start(out=out_ap[t0:t0 + tl, :, :], in_=packs[:tl, :, :])

    nc.compile()
    return nc


def _prep_inputs(x, parameters, rtwts):
    x = np.ascontiguousarray(np.asarray(x, np.float32))
    parameters = np.ascontiguousarray(np.asarray(parameters, np.float32))
    rtwts = np.ascontiguousarray(np.asarray(rtwts, np.float32))
    NPAD = NC_CORES * G
    xp = np.zeros((NSTEP, NPAD, 3), np.float32)
    xp[:, :NGRID] = x
    pp = np.full((NPAD, 12, MU), 0.5, np.float32)
    pp[:NGRID] = parameters
    rp = np.full((NPAD, 2), 0.5, np.float32)
    rp[:NGRID] = rtwts

    # physical params per (grid, mu): [NPAD, 12, MU]
    lo = PARA_SCALE[:, 0][None, :, None]
    hi = PARA_SCALE[:, 1][None, :, None]
    phys = lo + pp * (hi - lo)
    (BETA, FC, K0, K1, K2, LP, PPc, UZL, TT, CFMAX, CFR, CWH) = \
        [phys[:, j, :] for j in range(12)]

    Pfull, Tfull, Efull = xp[:, :, 0], xp[:, :, 1], xp[:, :, 2]  # [NSTEP, NPAD]

    wmean = np.zeros((P, GL), np.float32)
    for p in range(P):
        wmean[p, p // MU] = 1.0 / MU
    id128 = np.eye(P, dtype=np.float32)
    tk = (np.arange(LENF, dtype=np.float32) + 0.5).reshape(1, LENF)
    lntk = np.log(tk).astype(np.float32)

    def lanes(arr_g_mu, sl):
        # [192, MU] (grid-major) -> [P=GL*MU, GH] lane layout
        a = arr_g_mu[sl].reshape(GL, GH, MU).transpose(0, 2, 1)  # gl, mu, gh
        return np.ascontiguousarray(a.reshape(P, GH).astype(np.float32))

    in_maps = []
    for c in range(NC_CORES):
        sl = slice(c * G, (c + 1) * G)
        TTl = lanes(TT, sl); CFMAXl = lanes(CFMAX, sl); CFRl = lanes(CFR, sl)
        LPl = lanes(LP, sl); FCl = lanes(FC, sl)

        # [NSTEP, 192] -> [NSTEP, GL, 1, GH] -> broadcast mu -> [NSTEP, P, GH]
        def blanes(arr):
            a = arr[:, sl].reshape(NSTEP, GL, GH)
            a = np.broadcast_to(a[:, :, None, :], (NSTEP, GL, MU, GH))
            return a.reshape(NSTEP, P, GH)

        Pl, Tl, El = blanes(Pfull), blanes(Tfull), blanes(Efull)
        db = Tl - TTl[None]
        ge = (db >= 0.0).astype(np.float32)
        R = Pl * ge
        s = Pl - R
        mcap = np.maximum(CFMAXl[None] * db, 0.0)
        ccap = np.maximum(-(CFRl * CFMAXl)[None] * db, 0.0)
        ab = s - mcap + ccap
        NPADT = NSTEP + TC
        ab48 = np.zeros((NPADT, P, 48), np.float32)
        ab48[:NSTEP, :, 0:GH] = ab
        ab48[:NSTEP, :, GH:48] = s
        F = np.minimum(El / (LPl * FCl)[None], 1.0).astype(np.float32)
        Fc = (1.0 - F).astype(np.float32)

        def padt(arr):
            out = np.zeros((NPADT, P, GH), np.float32)
            out[:NSTEP] = arr
            return out

        parc = pp[sl].reshape(GL, GH, 12, MU).transpose(0, 3, 2, 1)
        in_maps.append({
            "ab48": np.ascontiguousarray(ab48),
            "rb": np.ascontiguousarray(padt(R)),
            "fcb": np.ascontiguousarray(padt(Fc)),
            "fb": np.ascontiguousarray(padt(F)),
            "eb": np.ascontiguousarray(padt(El)),
            "par": np.ascontiguousarray(parc.reshape(P, 12, GH)),
            "rt": np.ascontiguousarray(rp[sl]),
            "wmean": wmean, "id128": id128, "lntk": lntk, "tk": tk,
        })
    return in_maps


def kernel(x, parameters, rtwts, mu, _want_trace=False):
    assert int(mu) == MU
    from concourse.bass_utils import run_bass_kernel_spmd
    if "prog" not in _PROGRAM_CACHE:
        _PROGRAM_CACHE["prog"] = _build_program()
    nc = _PROGRAM_CACHE["prog"]
    in_maps = _prep_inputs(x, parameters, rtwts)
    res = run_bass_kernel_spmd(nc, in_maps, core_ids=list(range(NC_CORES)),
                               trace=_want_trace)
    outs = [r["out"] for r in res.results]  # each [730, 192, 5]
    full = np.concatenate(outs, axis=1)[:, :NGRID, :]
    if _want_trace:
        _PROGRAM_CACHE["last_results"] = res
    return np.ascontiguousarray(full.astype(np.float32))


# revision 24
# speedup vs baseline: 1.0013x; 1.0013x over previous
"""HBV hydrology model (nn_HBVMul) Trainium2 Bass kernel — v2.

Sharding: data-parallel over the 1500-grid axis across 8 cores (192 grids/core,
padded to 1536). Per-core lane layout: partition p = g_lo*16 + mu (g_lo in 0..7,
mu in 0..15), free dim g_hi in 0..23; local grid = g_lo*24 + g_hi.

v2 restructure (validated in numpy against the jax reference and on HW):
  - Exogenous per-step series (snowfall s, melt/refreeze budget ab, rain R,
    evap factors F = min(E/(LP*FC), 1) and Fc = 1-F) are computed host-side
    and DMA'd in; they are pure elementwise transforms of the inputs.
  - Activation-table pin: Ln/Exp forced into the shared
    natural_log_exp_and_others table (else the placement pass reloads the
    act table on every Ln<->Exp switch, ~1.3us twice per step).
  - Software-pipelined step cycle (~26 ops): cycle t runs the soil tail of
    step t-1 (rech..d2 via d2 = max(SMc*Fc, SMc-E), ET = min(SMc*F, E)
    exactly off the critical loop), the soil head of step t (m0 -> Act ln ->
    w1 -> Act exp), snow for step t, and the SUZ chain for step t-2 —
    ordered so every RAW dependency has >=1 unrelated op between producer
    and consumer (dependent back-to-back DVE ops cost ~210ns vs ~122ns) and
    both Act round trips have ~7 filler slots.
  - Inputs split into halves: h1 of chunk ci+1 is DMA-prefetched mid-body
    through +TC-shifted dram views (tensors padded by TC rows); the post
    pass (Q1/Q2/Qs on Pool, folded SLZ scan z' = (PERC+z)*(1-k2) on DVE,
    mu-mean matmuls on PE, staging copies on Act) runs in two halves so it
    overlaps the step loop.
  - Pool HW ISA only supports TT add/sub/mult (+ TensorScalar): no
    TensorScalarPtr/min/max there, so the state chains all stay on DVE.
"""

import os
import sys
import numpy as np

sys.path.insert(0, "/opt/trn_rl_repo")

NSTEP, NGRID, MU, LENF = 730, 1500, 16, 15
PRECS = 1e-5
NC_CORES = 8
G = 192          # grids per core
GL, GH = 8, 24   # g_lo x g_hi split of the 192 grids
P = 128          # partitions = GL * MU
NCH, TC = 10, 73  # time chunks
TSUB = [(0, 19), (19, 18), (37, 18), (55, 18)]  # mu-mean matmul sub-slices

PARA_SCALE = np.array([[1, 6], [50, 1000], [0.05, 0.9], [0.01, 0.5], [0.001, 0.2],
                       [0.2, 1], [0, 10], [0, 100], [-2.5, 2.5], [0.5, 10],
                       [0, 0.1], [0, 0.2]], dtype=np.float32)
ROUT_SCALE = np.array([[0, 2.9], [0, 6.5]], dtype=np.float32)

_PROGRAM_CACHE = {}


def _build_program():
    import concourse.bacc as bacc
    import concourse.bass as bass
    import concourse.tile as tile
    import concourse.mybir as mybir
    from concourse.bass import ts

    dt = mybir.dt
    Alu = mybir.AluOpType
    Act = mybir.ActivationFunctionType

    # Pin Ln/Exp to the combined activation table (json index 6,
    # natural_log_exp_and_others). The table-load placement pass picks, per
    # activation, some table containing its function; with Ln in
    # natural_log (5) and Exp in exp_and_others (0) it reloads the table on
    # every Ln<->Exp switch (~1.3us each, twice per step). Hiding Ln/Exp
    # from all other tables (ids/order unchanged, so the emitted
    # act_func_set_id stays valid) forces the shared table and hoists the
    # single load out of the loop.
    from concourse.hw_specs import get_activation_tables as _gat

    def _gat_pinned(arch):
        tabs = _gat(arch)
        keep = "natural_log_exp_and_others"
        drop = {mybir.ActivationFunctionType.Ln, mybir.ActivationFunctionType.Exp}
        return {name: (funcs if name == keep else funcs - drop)
                for name, funcs in tabs.items()}

    bacc.get_activation_tables = _gat_pinned

    nc = bacc.Bacc("TRN2", target_bir_lowering=False, debug=False,
                   num_devices=NC_CORES)

    f32 = dt.float32
    # padded by TC rows so the mid-chunk prefetch of chunk ci+1 stays in range
    NPADT = NSTEP + TC
    ab_ap = nc.dram_tensor("ab48", [NPADT, P, 48], f32, kind="ExternalInput").ap()
    rb_ap = nc.dram_tensor("rb", [NPADT, P, GH], f32, kind="ExternalInput").ap()
    fcb_ap = nc.dram_tensor("fcb", [NPADT, P, GH], f32, kind="ExternalInput").ap()
    fb_ap = nc.dram_tensor("fb", [NPADT, P, GH], f32, kind="ExternalInput").ap()
    eb_ap = nc.dram_tensor("eb", [NPADT, P, GH], f32, kind="ExternalInput").ap()
    par_ap = nc.dram_tensor("par", [P, 12, GH], f32, kind="ExternalInput").ap()
    rt_ap = nc.dram_tensor("rt", [G, 2], f32, kind="ExternalInput").ap()
    wm_ap = nc.dram_tensor("wmean", [P, GL], f32, kind="ExternalInput").ap()
    id_ap = nc.dram_tensor("id128", [P, P], f32, kind="ExternalInput").ap()
    lntk_ap = nc.dram_tensor("lntk", [1, LENF], f32, kind="ExternalInput").ap()
    tk_ap = nc.dram_tensor("tk", [1, LENF], f32, kind="ExternalInput").ap()
    out_ap = nc.dram_tensor("out", [NSTEP, G, 5], f32, kind="ExternalOutput").ap()

    scr = {}
    for name in ["smqs", "smq0", "smq1", "smq2", "smet"]:
        scr[name] = nc.dram_tensor(name, [NSTEP, G], f32, kind="Internal").ap()

    with tile.TileContext(nc) as tc:
        from contextlib import ExitStack
        ctx = ExitStack()
        with ctx:
            consts = ctx.enter_context(tc.tile_pool(name="consts", bufs=1))
            chunk = ctx.enter_context(tc.tile_pool(name="chunk", bufs=1))
            step = ctx.enter_context(tc.tile_pool(name="step", bufs=4))
            post = ctx.enter_context(tc.tile_pool(name="post", bufs=2))
            psum = ctx.enter_context(tc.tile_pool(name="psum", bufs=2, space="PSUM"))

            V = nc.vector
            S = nc.scalar
            GP = nc.gpsimd

            # ---- Phase 0: parameters ----
            par_sb = consts.tile([P, 12, GH], f32)
            nc.sync.dma_start(out=par_sb[:], in_=par_ap)
            pp_ = {}
            # param indices: 0 BETA, 1 FC, 2 K0, 3 K1, 4 K2, 5 LP, 6 PERC,
            #                7 UZL, 8 TT, 9 CFMAX, 10 CFR, 11 CWH
            for j in (0, 1, 2, 3, 4, 6, 7, 11):
                pt = consts.tile([P, GH], f32, tag=f"par{j}", name=f"par{j}")
                lo, hi = float(PARA_SCALE[j, 0]), float(PARA_SCALE[j, 1])
                V.tensor_scalar(pt[:], par_sb[:, j, :], hi - lo, lo,
                                Alu.mult, Alu.add)
                pp_[j] = pt
            betab, FCb, k0b, k1b, k2b = pp_[0], pp_[1], pp_[2], pp_[3], pp_[4]
            ppb, uzlb, CWHb = pp_[6], pp_[7], pp_[11]

            def ctile(tag, w=GH):
                return consts.tile([P, w], f32, tag=tag, name=tag)

            kcb = ctile("kcb");     V.tensor_scalar(kcb[:], CWHb[:], 1.0, None, Alu.add)
            iFCb = ctile("ifc");    V.reciprocal(iFCb[:], FCb[:])
            omk1b = ctile("omk1");  V.tensor_scalar(omk1b[:], k1b[:], -1.0, 1.0, Alu.mult, Alu.add)
            aslzb = ctile("aslz");  V.tensor_scalar(aslzb[:], k2b[:], -1.0, 1.0, Alu.mult, Alu.add)
            ralz = ctile("ralz");   V.reciprocal(ralz[:], aslzb[:])
            kqb = ctile("kqb");     V.tensor_tensor(kqb[:], k2b[:], ralz[:], Alu.mult)

            wm_sb = consts.tile([P, GL], f32)
            nc.sync.dma_start(out=wm_sb[:], in_=wm_ap)
            id_sb = consts.tile([P, P], f32)
            nc.sync.dma_start(out=id_sb[:], in_=id_ap)

            # ---- states ----
            SW48 = consts.tile([P, 48], f32, tag="SW48", name="SW48")
            V.memset(SW48[:, 0:GH], 0.001)   # SP
            V.memset(SW48[:, GH:48], 0.002)  # W = SNOWPACK + MELTWATER
            d2s = consts.tile([P, GH], f32, tag="d2s", name="d2s")
            V.memset(d2s[:], 0.001)          # raw SM (clamp applied at use)
            SUZs = consts.tile([P, GH], f32, tag="SUZ", name="SUZ")
            V.memset(SUZs[:], 0.001)
            SLZl = consts.tile([P, GH], f32, tag="SLZ", name="SLZ")
            V.memset(SLZl[:], 0.001)

            # ---- chunk buffers ----
            IH = 37   # input half split: h1=[0,IH) prefetched, h2=[IH,TC)
            OH = 34   # output half split for the mid-chunk post pass

            def cbuf(tag, n, w=GH):
                return chunk.tile([P, n, w], f32, tag=tag, name=tag)
            ABh = [cbuf("ABh1", IH, 48), cbuf("ABh2", TC - IH, 48)]
            Rh = [cbuf("Rh1", IH), cbuf("Rh2", TC - IH)]
            Fch = [cbuf("Fch1", IH), cbuf("Fch2", TC - IH)]
            Fh = [cbuf("Fh1", IH), cbuf("Fh2", TC - IH)]
            Eh = [cbuf("Eh1", IH), cbuf("Eh2", TC - IH)]

            def hv(bufs, t):  # half-split view of input series at step t
                return bufs[0][:, t, :] if t < IH else bufs[1][:, t - IH, :]

            ib = cbuf("ib", TC); rub = cbuf("rub", TC)
            PERCb = cbuf("PERCb", TC); Q0b = cbuf("Q0b", TC); yb = cbuf("yb", TC)
            ETb = cbuf("ETb", TC); Q1b = cbuf("Q1b", TC)
            zb = cbuf("zb", TC)
            Q2b = cbuf("Q2b", TC); Qsb = cbuf("Qsb", TC)

            def bcr(t, n):  # broadcast [P, GH] param over n steps
                return t[:, None, :].to_broadcast([P, n, GH])

            scr_views = {k: v.rearrange("(c t) (gl gh) -> c gl t gh", c=NCH, gl=GL)
                         for k, v in scr.items()}

            def stile(tag, w=GH):
                return step.tile([P, w], f32, tag=tag, name=tag)

            def emit_post(ci, h0, hn, subs):
                """Batched post for output positions [h0, h0+hn): Q1/q2t/Q2/Qs
                on Pool, SLZ scans on DVE, mu-means on PE."""
                sl = slice(h0, h0 + hn)
                GP.tensor_tensor(Q1b[:, sl, :], bcr(k1b, hn), yb[:, sl, :], Alu.mult)
                for g in range(GH):
                    # z' = (PERC[t] + z) * (1-k2): SLZ update as one scan
                    V.tensor_tensor_scan(
                        zb[:, sl, g], PERCb[:, sl, g],
                        aslzb[:, g:g + 1].to_broadcast([P, hn]),
                        SLZl[:, g:g + 1], Alu.add, Alu.mult)
                V.tensor_copy(out=SLZl[:], in_=zb[:, h0 + hn - 1, :])
                GP.tensor_tensor(Q2b[:, sl, :], zb[:, sl, :], bcr(kqb, hn), Alu.mult)
                GP.tensor_tensor(Qsb[:, sl, :], Q0b[:, sl, :], Q1b[:, sl, :], Alu.add)
                GP.tensor_tensor(Qsb[:, sl, :], Qsb[:, sl, :], Q2b[:, sl, :], Alu.add)
                for buf, name in ((Qsb, "smqs"), (Q0b, "smq0"), (Q1b, "smq1"),
                                  (Q2b, "smq2"), (ETb, "smet")):
                    for (t0, tl) in subs:
                        ps = psum.tile([GL, 20 * GH], f32, tag="msum", name="msum")
                        rhs = buf[:, t0:t0 + tl, :].rearrange("p t g -> p (t g)")
                        nc.tensor.matmul(ps[:, :tl * GH], wm_sb[:], rhs,
                                         start=True, stop=True)
                        stg = post.tile([GL, 20 * GH], f32, tag="mstg", name="mstg")
                        S.copy(stg[:, :tl * GH], ps[:, :tl * GH])
                        dst = scr_views[name][ci][:, t0:t0 + tl, :]
                        nc.sync.dma_start(
                            out=dst,
                            in_=stg[:, :tl * GH].rearrange("m (t g) -> m t g", g=GH))

            SUBS_H1 = [(0, 17), (17, 17)]
            SUBS_H2 = [(OH, 20), (OH + 20, TC - OH - 20)]
            in_srcs = ((ABh, ab_ap), (Rh, rb_ap), (Fch, fcb_ap),
                       (Fh, fb_ap), (Eh, eb_ap))
            # shifted base views for the mid-chunk prefetch of chunk ci+1
            in_shift = {id(bufs): src[TC:TC + NSTEP] for bufs, src in in_srcs}

            # preload chunk 0 h1 before the loop
            for bufs, src in in_srcs:
                sl = src[0:IH].rearrange("t p g -> p t g")
                nc.sync.dma_start(out=bufs[0][:], in_=sl)

            # Software-pipelined step cycle. Cycle t computes, interleaved so
            # every RAW dependency has >=1 unrelated op between producer and
            # consumer (dependent back-to-back DVE ops cost ~210ns vs ~122ns):
            #   soil tail of step t-1 (rech..d2/ET/ru, using sw issued last
            #   cycle, so the Act exp round-trip has a full cycle of slack),
            #   soil head of step t (m0 -> ln -> w1 -> exp),
            #   snow for step t, and the full SUZ chain for step t-2.
            with tc.For_i(0, NCH, 1) as ci:
                # current chunk h2 loads (h1 prefetched during the previous
                # iteration; chunk 0 h1 preloaded above)
                for bufs, src in in_srcs:
                    sl = src[ts(ci, TC)][IH:].rearrange("t p g -> p t g")
                    nc.sync.dma_start(out=bufs[1][:], in_=sl)

                def head_ops(t, UW):
                    """soil head + snow for step t; returns (UW_next, sw)."""
                    # emitted inline by emit_cycle; only used for t=0 prologue
                    m0 = stile("m0")
                    V.scalar_tensor_tensor(m0[:], d2s[:], PRECS, iFCb[:],
                                           Alu.max, Alu.mult)
                    S.activation(m0[:], m0[:], Act.Ln); l = m0
                    V.scalar_tensor_tensor(SW48[:, 0:GH], UW[:, 0:GH], 0.0,
                                           UW[:, GH:48], Alu.max, Alu.min)
                    v_ = stile("v_")
                    V.tensor_tensor(v_[:], kcb[:], SW48[:, 0:GH], Alu.mult)
                    V.tensor_tensor(SW48[:, GH:48], v_[:], UW[:, GH:48], Alu.min)
                    w1 = stile("w1"); V.tensor_tensor(w1[:], betab[:], l[:], Alu.mult)
                    S.activation(w1[:], w1[:], Act.Exp); sw = w1
                    q = stile("q")
                    V.tensor_tensor(q[:], UW[:, GH:48], SW48[:, GH:48], Alu.subtract)
                    V.tensor_tensor(ib[:, t, :], hv(Rh, t), q[:], Alu.add)
                    UW2 = stile("UW", 48)
                    V.tensor_tensor(UW2[:], SW48[:], hv(ABh, t + 1), Alu.add)
                    return UW2, sw

                # t=0 prologue
                UW = stile("UW", 48)
                V.tensor_tensor(UW[:], SW48[:], hv(ABh, 0), Alu.add)
                UW, sw_prev = head_ops(0, UW)

                def tail_ops(u, sw_u, zstep):
                    """soil tail for step u interleaved with the SUZ chain for
                    step zstep (None to skip); emits in dependency-spaced
                    order. Returns nothing; updates d2s/SUZs states."""
                    z = zstep is not None
                    rech = stile("rech")
                    V.tensor_tensor(rech[:], sw_u[:], ib[:, u, :], Alu.mult)
                    if z:
                        uu = stile("uu")
                        V.tensor_tensor(uu[:], SUZs[:], rub[:, zstep, :], Alu.add)
                    SMa = stile("SMa")
                    V.scalar_tensor_tensor(SMa[:], d2s[:], PRECS, ib[:, u, :],
                                           Alu.max, Alu.add)
                    if z:
                        V.tensor_tensor(PERCb[:, zstep, :], uu[:], ppb[:], Alu.min)
                    SMb = stile("SMb")
                    V.tensor_tensor(SMb[:], SMa[:], rech[:], Alu.subtract)
                    SMc = stile("SMc")
                    V.tensor_tensor(SMc[:], SMb[:], FCb[:], Alu.min)
                    if z:
                        vv = stile("vv")
                        V.tensor_tensor(vv[:], uu[:], PERCb[:, zstep, :], Alu.subtract)
                    a1 = stile("a1")
                    V.tensor_tensor(a1[:], SMc[:], hv(Fch, u), Alu.mult)
                    a2 = stile("a2")
                    V.tensor_tensor(a2[:], SMc[:], hv(Eh, u), Alu.subtract)
                    if z:
                        t1 = stile("t1")
                        V.tensor_tensor(t1[:], vv[:], uzlb[:], Alu.min)
                    V.tensor_tensor(d2s[:], a1[:], a2[:], Alu.max)
                    V.tensor_tensor(rub[:, u, :], SMa[:], SMc[:], Alu.subtract)
                    if z:
                        x0 = stile("x0")
                        V.tensor_tensor(x0[:], vv[:], t1[:], Alu.subtract)
                    zF = stile("zF")
                    V.tensor_tensor(zF[:], SMc[:], hv(Fh, u), Alu.mult)
                    V.tensor_tensor(ETb[:, u, :], zF[:], hv(Eh, u), Alu.min)
                    if z:
                        V.tensor_tensor(Q0b[:, zstep, :], k0b[:], x0[:], Alu.mult)
                        V.tensor_tensor(yb[:, zstep, :], vv[:], Q0b[:, zstep, :],
                                        Alu.subtract)
                        V.tensor_tensor(SUZs[:], omk1b[:], yb[:, zstep, :], Alu.mult)

                for t in range(1, TC):
                    u, v = t - 1, t - 2  # soil-tail step, suz step
                    z = v >= 0
                    # 26-slot cycle; SM loop (rech->SMb->SMc->a1/a2->d2->m0->
                    # ln->w1->exp->wrap rech) saturates it with 7-slot gaps
                    # around both Act round trips.
                    # s0 rech(u) = (sw + 0)*i; the zero gate tile is
                    # written late in the previous cycle so the scheduler
                    # cannot enqueue rech before the exp result is near
                    rech = stile("rech")
                    V.tensor_tensor(rech[:], sw_prev[:], ib[:, u, :], Alu.mult)
                    # s1 SMa(u)
                    SMa = stile("SMa")
                    V.scalar_tensor_tensor(SMa[:], d2s[:], PRECS, ib[:, u, :],
                                           Alu.max, Alu.add)
                    # s2 z1: uu = SUZ + ru[v]
                    if z:
                        uu = stile("uu")
                        V.tensor_tensor(uu[:], SUZs[:], rub[:, v, :], Alu.add)
                    # s3 SMb(u)
                    SMb = stile("SMb")
                    V.tensor_tensor(SMb[:], SMa[:], rech[:], Alu.subtract)
                    # s4 z2: PERC
                    if z:
                        V.tensor_tensor(PERCb[:, v, :], uu[:], ppb[:], Alu.min)
                    # s5 SMc(u)
                    SMc = stile("SMc")
                    V.tensor_tensor(SMc[:], SMb[:], FCb[:], Alu.min)
                    # s6 z3: vv
                    if z:
                        vv = stile("vv")
                        V.tensor_tensor(vv[:], uu[:], PERCb[:, v, :], Alu.subtract)
                    # s7 a1 = SMc*Fc[u]
                    a1 = stile("a1")
                    V.tensor_tensor(a1[:], SMc[:], hv(Fch, u), Alu.mult)
                    # s8 a2 = SMc - E[u]
                    a2 = stile("a2")
                    V.tensor_tensor(a2[:], SMc[:], hv(Eh, u), Alu.subtract)
                    # s9 z4: t1
                    if z:
                        t1 = stile("t1")
                        V.tensor_tensor(t1[:], vv[:], uzlb[:], Alu.min)
                    # s10 d2 = max(a1, a2)
                    V.tensor_tensor(d2s[:], a1[:], a2[:], Alu.max)
                    # s11 z5: x0
                    if z:
                        x0 = stile("x0")
                        V.tensor_tensor(x0[:], vv[:], t1[:], Alu.subtract)
                    # s12 m0(t) -> ln
                    m0 = stile("m0")
                    V.scalar_tensor_tensor(m0[:], d2s[:], PRECS, iFCb[:],
                                           Alu.max, Alu.mult)
                    S.activation(m0[:], m0[:], Act.Ln); l = m0
                    # s13 z6: Q0
                    if z:
                        V.tensor_tensor(Q0b[:, v, :], k0b[:], x0[:], Alu.mult)
                    # s14 n2: SP' (snow stays on DVE: Pool HW lacks
                    # TensorScalarPtr and min/max TensorTensor)
                    V.scalar_tensor_tensor(SW48[:, 0:GH], UW[:, 0:GH], 0.0,
                                           UW[:, GH:48], Alu.max, Alu.min)
                    # s15 z7: y
                    if z:
                        V.tensor_tensor(yb[:, v, :], vv[:], Q0b[:, v, :], Alu.subtract)
                    # s16 n3: v = kc*SP'
                    v_ = stile("v_")
                    V.tensor_tensor(v_[:], kcb[:], SW48[:, 0:GH], Alu.mult)
                    # s17 z8: SUZ'
                    if z:
                        V.tensor_tensor(SUZs[:], omk1b[:], yb[:, v, :], Alu.mult)
                    # s18 n4: W'
                    V.tensor_tensor(SW48[:, GH:48], v_[:], UW[:, GH:48], Alu.min)
                    # s19 w1 = (l + 0)*beta -> exp (gated like rech)
                    w1 = stile("w1")
                    V.tensor_tensor(w1[:], betab[:], l[:], Alu.mult)
                    S.activation(w1[:], w1[:], Act.Exp); sw = w1
                    # s20 n5: q
                    q = stile("q")
                    V.tensor_tensor(q[:], UW[:, GH:48], SW48[:, GH:48], Alu.subtract)
                    # s21 zF(u) = SMc * F[u]   (exact ET path, off the SM loop)
                    zF = stile("zF")
                    V.tensor_tensor(zF[:], SMc[:], hv(Fh, u), Alu.mult)
                    # s22 n6: i
                    V.tensor_tensor(ib[:, t, :], hv(Rh, t), q[:], Alu.add)
                    # s23 ET(u) = min(zF, E[u])
                    V.tensor_tensor(ETb[:, u, :], zF[:], hv(Eh, u), Alu.min)
                    # s24 n1+: UW(t+1)
                    if t < TC - 1:
                        UW = stile("UW", 48)
                        V.tensor_tensor(UW[:], SW48[:], hv(ABh, t + 1), Alu.add)
                    # s25 ru(u)
                    V.tensor_tensor(rub[:, u, :], SMa[:], SMc[:], Alu.subtract)
                    sw_prev = sw

                    if t == 36:
                        # prefetch next chunk's AB/R h1 (their h1 reads end at
                        # cycle 36); overlaps the remaining cycles
                        for bufs in (ABh, Rh):
                            sl = in_shift[id(bufs)][ts(ci, TC)][0:IH] \
                                .rearrange("t p g -> p t g")
                            nc.sync.dma_start(out=bufs[0][:], in_=sl)
                        # first-half post (outputs complete through OH-1)
                        emit_post(ci, 0, OH, SUBS_H1)
                    elif t == 37:
                        # Fc/E reads lag one cycle (index u=t-1): h1 clear now
                        for bufs in (Fch, Fh, Eh):
                            sl = in_shift[id(bufs)][ts(ci, TC)][0:IH] \
                                .rearrange("t p g -> p t g")
                            nc.sync.dma_start(out=bufs[0][:], in_=sl)

                # epilogue: soil tail for TC-1 with suz TC-2, then suz TC-1
                tail_ops(TC - 1, sw_prev, TC - 2)
                vv = stile("vv"); uu = stile("uu")
                s = TC - 1
                V.tensor_tensor(uu[:], SUZs[:], rub[:, s, :], Alu.add)
                V.tensor_tensor(PERCb[:, s, :], uu[:], ppb[:], Alu.min)
                V.tensor_tensor(vv[:], uu[:], PERCb[:, s, :], Alu.subtract)
                t1 = stile("t1"); V.tensor_tensor(t1[:], vv[:], uzlb[:], Alu.min)
                x0 = stile("x0"); V.tensor_tensor(x0[:], vv[:], t1[:], Alu.subtract)
                V.tensor_tensor(Q0b[:, s, :], k0b[:], x0[:], Alu.mult)
                V.tensor_tensor(yb[:, s, :], vv[:], Q0b[:, s, :], Alu.subtract)
                V.tensor_tensor(SUZs[:], omk1b[:], yb[:, s, :], Alu.mult)

                # second-half post
                emit_post(ci, OH, TC - OH, SUBS_H2)

            # ---- finale: routing conv + output assembly ----
            gparts = [(0, 128), (128, 64)]
            wtaps = []
            for (p0, pl) in gparts:
                rts = post.tile([pl, 2], f32, tag=f"rts{p0}", name=f"rts{p0}")
                nc.sync.dma_start(out=rts[:], in_=rt_ap[p0:p0 + pl, :])
                am1 = post.tile([pl, 1], f32, tag=f"am1{p0}", name=f"am1{p0}")
                V.tensor_scalar(am1[:], rts[:, 0:1], 2.9, 0.0, Alu.mult, Alu.max)
                V.tensor_scalar(am1[:], am1[:], -0.9, None, Alu.add)
                th = post.tile([pl, 1], f32, tag=f"th{p0}", name=f"th{p0}")
                V.tensor_scalar(th[:], rts[:, 1:2], 6.5, 0.0, Alu.mult, Alu.max)
                V.tensor_scalar(th[:], th[:], 0.5, None, Alu.add)
                ivt = post.tile([pl, 1], f32, tag=f"ivt{p0}", name=f"ivt{p0}")
                V.reciprocal(ivt[:], th[:])
                lnt = post.tile([pl, LENF], f32, tag=f"lnt{p0}", name=f"lnt{p0}")
                nc.sync.dma_start(out=lnt[:], in_=lntk_ap.to_broadcast([pl, LENF]))
                tkb = post.tile([pl, LENF], f32, tag=f"tkb{p0}", name=f"tkb{p0}")
                nc.sync.dma_start(out=tkb[:], in_=tk_ap.to_broadcast([pl, LENF]))
                e1 = post.tile([pl, LENF], f32, tag=f"e1{p0}", name=f"e1{p0}")
                V.tensor_scalar(e1[:], lnt[:], am1[:, 0:1], None, Alu.mult)
                e2 = post.tile([pl, LENF], f32, tag=f"e2{p0}", name=f"e2{p0}")
                V.tensor_scalar(e2[:], tkb[:], ivt[:, 0:1], None, Alu.mult)
                V.tensor_tensor(e1[:], e1[:], e2[:], Alu.subtract)
                vt = post.tile([pl, LENF], f32, tag=f"vt{p0}", name=f"vt{p0}")
                S.activation(vt[:], e1[:], Act.Exp)
                ssum = post.tile([pl, 1], f32, tag=f"ss{p0}", name=f"ss{p0}")
                V.tensor_reduce(ssum[:], vt[:], mybir.AxisListType.X, Alu.add)
                rs = post.tile([pl, 1], f32, tag=f"rs{p0}", name=f"rs{p0}")
                V.reciprocal(rs[:], ssum[:])
                wt = post.tile([pl, LENF], f32, tag=f"wt{p0}", name=f"wt{p0}")
                V.tensor_scalar(wt[:], vt[:], rs[:, 0:1], None, Alu.mult)
                wtaps.append(wt)

            tblocks = [(i * 128, min(128, NSTEP - i * 128)) for i in range(6)]

            # transposed-load Qsimave into [grid, time] padded tiles
            xps = []
            for (p0, pl) in gparts:
                xp = post.tile([pl, LENF - 1 + NSTEP], f32, tag=f"xp{p0}", name=f"xp{p0}")
                V.memset(xp[:], 0.0)
                xps.append(xp)
            for (t0, tl) in tblocks:
                ld = post.tile([128, G], f32, tag="qsld", name="qsld")
                nc.sync.dma_start(out=ld[:tl, :], in_=scr["smqs"][t0:t0 + tl, :])
                for xi, (p0, pl) in enumerate(gparts):
                    pst = psum.tile([pl, 128], f32, tag="ptr", name="ptr")
                    nc.tensor.transpose(pst[:, :tl], ld[:tl, p0:p0 + pl],
                                        id_sb[:tl, :tl])
                    V.tensor_copy(out=xps[xi][:, LENF - 1 + t0:LENF - 1 + t0 + tl],
                                  in_=pst[:, :tl])

            accs = []
            for xi, (p0, pl) in enumerate(gparts):
                acc = post.tile([pl, NSTEP], f32, tag=f"acc{p0}", name=f"acc{p0}")
                acc2 = post.tile([pl, NSTEP], f32, tag=f"acc2{p0}", name=f"acc2{p0}")
                xp, wt = xps[xi], wtaps[xi]
                V.tensor_scalar(acc[:], xp[:, LENF - 1:LENF - 1 + NSTEP],
                                wt[:, 0:1], None, Alu.mult)
                cur, nxt = acc, acc2
                for k in range(1, LENF):
                    V.scalar_tensor_tensor(
                        nxt[:], xp[:, LENF - 1 - k:LENF - 1 - k + NSTEP],
                        wt[:, k:k + 1], cur[:], Alu.mult, Alu.add)
                    cur, nxt = nxt, cur
                accs.append(cur)

            for (t0, tl) in tblocks:
                packs = post.tile([128, G, 5], f32, tag="packs", name="packs")
                for j, name in enumerate(["smq0", "smq1", "smq2", "smet"]):
                    ld = post.tile([128, G], f32, tag=f"mld{j}", name=f"mld{j}")
                    nc.sync.dma_start(out=ld[:tl, :], in_=scr[name][t0:t0 + tl, :])
                    V.tensor_copy(out=packs[:tl, :, j + 1], in_=ld[:tl, :])
                for xi, (p0, pl) in enumerate(gparts):
                    pst = psum.tile([128, 128], f32, tag="ptr2", name="ptr2")
                    nc.tensor.transpose(pst[:tl, :pl], accs[xi][:, t0:t0 + tl],
                                        id_sb[:pl, :pl])
                    V.tensor_copy(out=packs[:tl, p0:p0 + pl, 0], in_=pst[:tl, :pl])
                nc.sync.dma_start(out=out_ap[t0:t0 + tl, :, :], in_=packs[:tl, :, :])

    nc.compile()
    return nc


def _prep_inputs(x, parameters, rtwts):
    x = np.ascontiguousarray(np.asarray(x, np.float32))
    parameters = np.ascontiguousarray(np.asarray(parameters, np.float32))
    rtwts = np.ascontiguousarray(np.asarray(rtwts, np.float32))
    NPAD = NC_CORES * G
    xp = np.zeros((NSTEP, NPAD, 3), np.float32)
    xp[:, :NGRID] = x
    pp = np.full((NPAD, 12, MU), 0.5, np.float32)
    pp[:NGRID] = parameters
    rp = np.full((NPAD, 2), 0.5, np.float32)
    rp[:NGRID] = rtwts

    # physical params per (grid, mu): [NPAD, 12, MU]
    lo = PARA_SCALE[:, 0][None, :, None]
    hi = PARA_SCALE[:, 1][None, :, None]
    phys = lo + pp * (hi - lo)
    (BETA, FC, K0, K1, K2, LP, PPc, UZL, TT, CFMAX, CFR, CWH) = \
        [phys[:, j, :] for j in range(12)]

    Pfull, Tfull, Efull = xp[:, :, 0], xp[:, :, 1], xp[:, :, 2]  # [NSTEP, NPAD]

    wmean = np.zeros((P, GL), np.float32)
    for p in range(P):
        wmean[p, p // MU] = 1.0 / MU
    id128 = np.eye(P, dtype=np.float32)
    tk = (np.arange(LENF, dtype=np.float32) + 0.5).reshape(1, LENF)
    lntk = np.log(tk).astype(np.float32)

    def lanes(arr_g_mu, sl):
        # [192, MU] (grid-major) -> [P=GL*MU, GH] lane layout
        a = arr_g_mu[sl].reshape(GL, GH, MU).transpose(0, 2, 1)  # gl, mu, gh
        return np.ascontiguousarray(a.reshape(P, GH).astype(np.float32))

    in_maps = []
    for c in range(NC_CORES):
        sl = slice(c * G, (c + 1) * G)
        TTl = lanes(TT, sl); CFMAXl = lanes(CFMAX, sl); CFRl = lanes(CFR, sl)
        LPl = lanes(LP, sl); FCl = lanes(FC, sl)

        # [NSTEP, 192] -> [NSTEP, GL, 1, GH] -> broadcast mu -> [NSTEP, P, GH]
        def blanes(arr):
            a = arr[:, sl].reshape(NSTEP, GL, GH)
            a = np.broadcast_to(a[:, :, None, :], (NSTEP, GL, MU, GH))
            return a.reshape(NSTEP, P, GH)

        Pl, Tl, El = blanes(Pfull), blanes(Tfull), blanes(Efull)
        db = Tl - TTl[None]
        ge = (db >= 0.0).astype(np.float32)
        R = Pl * ge
        s = Pl - R
        mcap = np.maximum(CFMAXl[None] * db, 0.0)
        ccap = np.maximum(-(CFRl * CFMAXl)[None] * db, 0.0)
        ab = s - mcap + ccap
        NPADT = NSTEP + TC
        ab48 = np.zeros((NPADT, P, 48), np.float32)
        ab48[:NSTEP, :, 0:GH] = ab
        ab48[:NSTEP, :, GH:48] = s
        F = np.minimum(El / (LPl * FCl)[None], 1.0).astype(np.float32)
        Fc = (1.0 - F).astype(np.float32)

        def padt(arr):
            out = np.zeros((NPADT, P, GH), np.float32)
            out[:NSTEP] = arr
            return out

        parc = pp[sl].reshape(GL, GH, 12, MU).transpose(0, 3, 2, 1)
        in_maps.append({
            "ab48": np.ascontiguousarray(ab48),
            "rb": np.ascontiguousarray(padt(R)),
            "fcb": np.ascontiguousarray(padt(Fc)),
            "fb": np.ascontiguousarray(padt(F)),
            "eb": np.ascontiguousarray(padt(El)),
            "par": np.ascontiguousarray(parc.reshape(P, 12, GH)),
            "rt": np.ascontiguousarray(rp[sl]),
            "wmean": wmean, "id128": id128, "lntk": lntk, "tk": tk,
        })
    return in_maps


def kernel(x, parameters, rtwts, mu, _want_trace=False):
    assert int(mu) == MU
    from concourse.bass_utils import run_bass_kernel_spmd
    if "prog" not in _PROGRAM_CACHE:
        _PROGRAM_CACHE["prog"] = _build_program()
    nc = _PROGRAM_CACHE["prog"]
    in_maps = _prep_inputs(x, parameters, rtwts)
    res = run_bass_kernel_spmd(nc, in_maps, core_ids=list(range(NC_CORES)),
                               trace=_want_trace)
    outs = [r["out"] for r in res.results]  # each [730, 192, 5]
    full = np.concatenate(outs, axis=1)[:, :NGRID, :]
    if _want_trace:
        _PROGRAM_CACHE["last_results"] = res
    return np.ascontiguousarray(full.astype(np.float32))


# revision 25
# speedup vs baseline: 1.1500x; 1.1485x over previous
"""HBV hydrology model (nn_HBVMul) Trainium2 Bass kernel — v2.

Sharding: data-parallel over the 1500-grid axis across 8 cores (192 grids/core,
padded to 1536). Per-core lane layout: partition p = g_lo*16 + mu (g_lo in 0..7,
mu in 0..15), free dim g_hi in 0..23; local grid = g_lo*24 + g_hi.

v2 restructure (validated in numpy against the jax reference and on HW):
  - Exogenous per-step series (snowfall s, melt/refreeze budget ab, rain R,
    evap factors F = min(E/(LP*FC), 1) and Fc = 1-F) are computed host-side
    and DMA'd in; they are pure elementwise transforms of the inputs.
  - Activation-table pin: Ln/Exp forced into the shared
    natural_log_exp_and_others table (else the placement pass reloads the
    act table on every Ln<->Exp switch, ~1.3us twice per step).
  - Software-pipelined step cycle (~26 ops): cycle t runs the soil tail of
    step t-1 (rech..d2 via d2 = max(SMc*Fc, SMc-E), ET = min(SMc*F, E)
    exactly off the critical loop), the soil head of step t (m0 -> Act ln ->
    w1 -> Act exp), snow for step t, and the SUZ chain for step t-2 —
    ordered so every RAW dependency has >=1 unrelated op between producer
    and consumer (dependent back-to-back DVE ops cost ~210ns vs ~122ns) and
    both Act round trips have ~7 filler slots.
  - Inputs split into halves: h1 of chunk ci+1 is DMA-prefetched mid-body
    through +TC-shifted dram views (tensors padded by TC rows); the post
    pass (Q1/Q2/Qs on Pool, folded SLZ scan z' = (PERC+z)*(1-k2) on DVE,
    mu-mean matmuls on PE, staging copies on Act) runs in two halves so it
    overlaps the step loop.
  - Pool HW ISA only supports TT add/sub/mult (+ TensorScalar): no
    TensorScalarPtr/min/max there, so the state chains all stay on DVE.
"""

import os
import sys
import numpy as np

sys.path.insert(0, "/opt/trn_rl_repo")

NSTEP, NGRID, MU, LENF = 730, 1500, 16, 15
PRECS = 1e-5
NC_CORES = 8
G = 192          # grids per core
GL, GH = 8, 24   # g_lo x g_hi split of the 192 grids
P = 128          # partitions = GL * MU
NCH, TC = 10, 73  # time chunks
TSUB = [(0, 19), (19, 18), (37, 18), (55, 18)]  # mu-mean matmul sub-slices

PARA_SCALE = np.array([[1, 6], [50, 1000], [0.05, 0.9], [0.01, 0.5], [0.001, 0.2],
                       [0.2, 1], [0, 10], [0, 100], [-2.5, 2.5], [0.5, 10],
                       [0, 0.1], [0, 0.2]], dtype=np.float32)
ROUT_SCALE = np.array([[0, 2.9], [0, 6.5]], dtype=np.float32)

_PROGRAM_CACHE = {}


def _build_program():
    import concourse.bacc as bacc
    import concourse.bass as bass
    import concourse.tile as tile
    import concourse.mybir as mybir
    from concourse.bass import ts

    dt = mybir.dt
    Alu = mybir.AluOpType
    Act = mybir.ActivationFunctionType

    # Pin Ln/Exp to the combined activation table (json index 6,
    # natural_log_exp_and_others). The table-load placement pass picks, per
    # activation, some table containing its function; with Ln in
    # natural_log (5) and Exp in exp_and_others (0) it reloads the table on
    # every Ln<->Exp switch (~1.3us each, twice per step). Hiding Ln/Exp
    # from all other tables (ids/order unchanged, so the emitted
    # act_func_set_id stays valid) forces the shared table and hoists the
    # single load out of the loop.
    from concourse.hw_specs import get_activation_tables as _gat

    def _gat_pinned(arch):
        tabs = _gat(arch)
        keep = "natural_log_exp_and_others"
        drop = {mybir.ActivationFunctionType.Ln, mybir.ActivationFunctionType.Exp}
        return {name: (funcs if name == keep else funcs - drop)
                for name, funcs in tabs.items()}

    bacc.get_activation_tables = _gat_pinned

    nc = bacc.Bacc("TRN2", target_bir_lowering=False, debug=False,
                   num_devices=NC_CORES)

    f32 = dt.float32
    # padded by TC rows so the mid-chunk prefetch of chunk ci+1 stays in range
    NPADT = NSTEP + TC
    ab_ap = nc.dram_tensor("ab48", [NPADT, P, 48], f32, kind="ExternalInput").ap()
    rb_ap = nc.dram_tensor("rb", [NPADT, P, GH], f32, kind="ExternalInput").ap()
    fcb_ap = nc.dram_tensor("fcb", [NPADT, P, GH], f32, kind="ExternalInput").ap()
    fb_ap = nc.dram_tensor("fb", [NPADT, P, GH], f32, kind="ExternalInput").ap()
    eb_ap = nc.dram_tensor("eb", [NPADT, P, GH], f32, kind="ExternalInput").ap()
    par_ap = nc.dram_tensor("par", [P, 12, GH], f32, kind="ExternalInput").ap()
    rt_ap = nc.dram_tensor("rt", [G, 2], f32, kind="ExternalInput").ap()
    wm_ap = nc.dram_tensor("wmean", [P, GL], f32, kind="ExternalInput").ap()
    id_ap = nc.dram_tensor("id128", [P, P], f32, kind="ExternalInput").ap()
    lntk_ap = nc.dram_tensor("lntk", [1, LENF], f32, kind="ExternalInput").ap()
    tk_ap = nc.dram_tensor("tk", [1, LENF], f32, kind="ExternalInput").ap()
    out_ap = nc.dram_tensor("out", [NSTEP, G, 5], f32, kind="ExternalOutput").ap()

    scr = {}
    for name in ["smqs", "smq0", "smq1", "smq2", "smet"]:
        scr[name] = nc.dram_tensor(name, [NSTEP, G], f32, kind="Internal").ap()

    with tile.TileContext(nc) as tc:
        from contextlib import ExitStack
        ctx = ExitStack()
        with ctx:
            consts = ctx.enter_context(tc.tile_pool(name="consts", bufs=1))
            chunk = ctx.enter_context(tc.tile_pool(name="chunk", bufs=1))
            step = ctx.enter_context(tc.tile_pool(name="step", bufs=4))
            post = ctx.enter_context(tc.tile_pool(name="post", bufs=2))
            psum = ctx.enter_context(tc.tile_pool(name="psum", bufs=2, space="PSUM"))

            V = nc.vector
            S = nc.scalar
            GP = nc.gpsimd

            # ---- Phase 0: parameters ----
            par_sb = consts.tile([P, 12, GH], f32)
            nc.sync.dma_start(out=par_sb[:], in_=par_ap)
            pp_ = {}
            # param indices: 0 BETA, 1 FC, 2 K0, 3 K1, 4 K2, 5 LP, 6 PERC,
            #                7 UZL, 8 TT, 9 CFMAX, 10 CFR, 11 CWH
            for j in (0, 1, 2, 3, 4, 6, 7, 11):
                pt = consts.tile([P, GH], f32, tag=f"par{j}", name=f"par{j}")
                lo, hi = float(PARA_SCALE[j, 0]), float(PARA_SCALE[j, 1])
                V.tensor_scalar(pt[:], par_sb[:, j, :], hi - lo, lo,
                                Alu.mult, Alu.add)
                pp_[j] = pt
            betab, FCb, k0b, k1b, k2b = pp_[0], pp_[1], pp_[2], pp_[3], pp_[4]
            ppb, uzlb, CWHb = pp_[6], pp_[7], pp_[11]

            def ctile(tag, w=GH):
                return consts.tile([P, w], f32, tag=tag, name=tag)

            kcb = ctile("kcb");     V.tensor_scalar(kcb[:], CWHb[:], 1.0, None, Alu.add)
            iFCb = ctile("ifc");    V.reciprocal(iFCb[:], FCb[:])
            omk1b = ctile("omk1");  V.tensor_scalar(omk1b[:], k1b[:], -1.0, 1.0, Alu.mult, Alu.add)
            aslzb = ctile("aslz");  V.tensor_scalar(aslzb[:], k2b[:], -1.0, 1.0, Alu.mult, Alu.add)
            ralz = ctile("ralz");   V.reciprocal(ralz[:], aslzb[:])
            kqb = ctile("kqb");     V.tensor_tensor(kqb[:], k2b[:], ralz[:], Alu.mult)

            wm_sb = consts.tile([P, GL], f32)
            nc.sync.dma_start(out=wm_sb[:], in_=wm_ap)
            id_sb = consts.tile([P, P], f32)
            nc.sync.dma_start(out=id_sb[:], in_=id_ap)

            # ---- states ----
            SW48 = consts.tile([P, 48], f32, tag="SW48", name="SW48")
            V.memset(SW48[:, 0:GH], 0.001)   # SP
            V.memset(SW48[:, GH:48], 0.002)  # W = SNOWPACK + MELTWATER
            d2s = consts.tile([P, GH], f32, tag="d2s", name="d2s")
            V.memset(d2s[:], 0.001)          # raw SM (clamp applied at use)
            SUZs = consts.tile([P, GH], f32, tag="SUZ", name="SUZ")
            V.memset(SUZs[:], 0.001)
            SLZl = consts.tile([P, GH], f32, tag="SLZ", name="SLZ")
            V.memset(SLZl[:], 0.001)

            # ---- chunk buffers ----
            IH = 37   # input half split: h1=[0,IH) prefetched, h2=[IH,TC)
            OH = 34   # output half split for the mid-chunk post pass

            def cbuf(tag, n, w=GH):
                return chunk.tile([P, n, w], f32, tag=tag, name=tag)
            ABh = [cbuf("ABh1", IH, 48), cbuf("ABh2", TC - IH, 48)]
            Rh = [cbuf("Rh1", IH), cbuf("Rh2", TC - IH)]
            Fch = [cbuf("Fch1", IH), cbuf("Fch2", TC - IH)]
            Fh = [cbuf("Fh1", IH), cbuf("Fh2", TC - IH)]
            Eh = [cbuf("Eh1", IH), cbuf("Eh2", TC - IH)]

            def hv(bufs, t):  # half-split view of input series at step t
                return bufs[0][:, t, :] if t < IH else bufs[1][:, t - IH, :]

            ib = cbuf("ib", TC); rub = cbuf("rub", TC)
            PERCb = cbuf("PERCb", TC); Q0b = cbuf("Q0b", TC); yb = cbuf("yb", TC)
            ETb = cbuf("ETb", TC); Q1b = cbuf("Q1b", TC)
            zb = cbuf("zb", TC)
            Q2b = cbuf("Q2b", TC); Qsb = cbuf("Qsb", TC)

            def bcr(t, n):  # broadcast [P, GH] param over n steps
                return t[:, None, :].to_broadcast([P, n, GH])

            scr_views = {k: v.rearrange("(c t) (gl gh) -> c gl t gh", c=NCH, gl=GL)
                         for k, v in scr.items()}

            def stile(tag, w=GH):
                return step.tile([P, w], f32, tag=tag, name=tag)

            def emit_post(ci, h0, hn, subs):
                """Batched post for output positions [h0, h0+hn): Q1/q2t/Q2/Qs
                on Pool, SLZ scans on DVE, mu-means on PE."""
                sl = slice(h0, h0 + hn)
                GP.tensor_tensor(Q1b[:, sl, :], bcr(k1b, hn), yb[:, sl, :], Alu.mult)
                for g in range(GH):
                    # z' = (PERC[t] + z) * (1-k2): SLZ update as one scan
                    V.tensor_tensor_scan(
                        zb[:, sl, g], PERCb[:, sl, g],
                        aslzb[:, g:g + 1].to_broadcast([P, hn]),
                        SLZl[:, g:g + 1], Alu.add, Alu.mult)
                V.tensor_copy(out=SLZl[:], in_=zb[:, h0 + hn - 1, :])
                GP.tensor_tensor(Q2b[:, sl, :], zb[:, sl, :], bcr(kqb, hn), Alu.mult)
                GP.tensor_tensor(Qsb[:, sl, :], Q0b[:, sl, :], Q1b[:, sl, :], Alu.add)
                GP.tensor_tensor(Qsb[:, sl, :], Qsb[:, sl, :], Q2b[:, sl, :], Alu.add)
                for buf, name in ((Qsb, "smqs"), (Q0b, "smq0"), (Q1b, "smq1"),
                                  (Q2b, "smq2"), (ETb, "smet")):
                    for (t0, tl) in subs:
                        ps = psum.tile([GL, 20 * GH], f32, tag="msum", name="msum")
                        rhs = buf[:, t0:t0 + tl, :].rearrange("p t g -> p (t g)")
                        nc.tensor.matmul(ps[:, :tl * GH], wm_sb[:], rhs,
                                         start=True, stop=True)
                        stg = post.tile([GL, 20 * GH], f32, tag="mstg", name="mstg")
                        S.copy(stg[:, :tl * GH], ps[:, :tl * GH])
                        dst = scr_views[name][ci][:, t0:t0 + tl, :]
                        nc.sync.dma_start(
                            out=dst,
                            in_=stg[:, :tl * GH].rearrange("m (t g) -> m t g", g=GH))

            SUBS_H1 = [(0, 17), (17, 17)]
            SUBS_H2 = [(OH, 20), (OH + 20, TC - OH - 20)]
            in_srcs = ((ABh, ab_ap), (Rh, rb_ap), (Fch, fcb_ap),
                       (Fh, fb_ap), (Eh, eb_ap))
            # base views for the mid-chunk prefetch of chunk ci+1
            in_shift = {id(bufs): srcap for bufs, srcap in in_srcs}

            # preload chunk 0 h1 before the loop
            for bufs, srcap in in_srcs:
                sl = srcap[0:IH].rearrange("t p g -> p t g")
                nc.sync.dma_start(out=bufs[0][:], in_=sl)

            # Software-pipelined step cycle. Cycle t computes, interleaved so
            # every RAW dependency has >=1 unrelated op between producer and
            # consumer (dependent back-to-back DVE ops cost ~210ns vs ~122ns):
            #   soil tail of step t-1 (rech..d2/ET/ru, using sw issued last
            #   cycle, so the Act exp round-trip has a full cycle of slack),
            #   soil head of step t (m0 -> ln -> w1 -> exp),
            #   snow for step t, and the full SUZ chain for step t-2.
            for ci in range(NCH):
                # current chunk h2 loads (h1 prefetched during the previous
                # iteration; chunk 0 h1 preloaded above)
                for bufs, srcap in in_srcs:
                    sl = srcap[ci * TC + IH:(ci + 1) * TC]                         .rearrange("t p g -> p t g")
                    nc.sync.dma_start(out=bufs[1][:], in_=sl)

                def head_ops(t, UW):
                    """soil head + snow for step t; returns (UW_next, sw)."""
                    # emitted inline by emit_cycle; only used for t=0 prologue
                    m0 = stile("m0")
                    V.scalar_tensor_tensor(m0[:], d2s[:], PRECS, iFCb[:],
                                           Alu.max, Alu.mult)
                    S.activation(m0[:], m0[:], Act.Ln); l = m0
                    V.scalar_tensor_tensor(SW48[:, 0:GH], UW[:, 0:GH], 0.0,
                                           UW[:, GH:48], Alu.max, Alu.min)
                    v_ = stile("v_")
                    V.tensor_tensor(v_[:], kcb[:], SW48[:, 0:GH], Alu.mult)
                    V.tensor_tensor(SW48[:, GH:48], v_[:], UW[:, GH:48], Alu.min)
                    w1 = stile("w1"); V.tensor_tensor(w1[:], betab[:], l[:], Alu.mult)
                    S.activation(w1[:], w1[:], Act.Exp); sw = w1
                    q = stile("q")
                    V.tensor_tensor(q[:], UW[:, GH:48], SW48[:, GH:48], Alu.subtract)
                    V.tensor_tensor(ib[:, t, :], hv(Rh, t), q[:], Alu.add)
                    UW2 = stile("UW", 48)
                    V.tensor_tensor(UW2[:], SW48[:], hv(ABh, t + 1), Alu.add)
                    return UW2, sw

                # t=0 prologue
                UW = stile("UW", 48)
                V.tensor_tensor(UW[:], SW48[:], hv(ABh, 0), Alu.add)
                UW, sw_prev = head_ops(0, UW)

                def tail_ops(u, sw_u, zstep):
                    """soil tail for step u interleaved with the SUZ chain for
                    step zstep (None to skip); emits in dependency-spaced
                    order. Returns nothing; updates d2s/SUZs states."""
                    z = zstep is not None
                    rech = stile("rech")
                    V.tensor_tensor(rech[:], sw_u[:], ib[:, u, :], Alu.mult)
                    if z:
                        uu = stile("uu")
                        V.tensor_tensor(uu[:], SUZs[:], rub[:, zstep, :], Alu.add)
                    SMa = stile("SMa")
                    V.scalar_tensor_tensor(SMa[:], d2s[:], PRECS, ib[:, u, :],
                                           Alu.max, Alu.add)
                    if z:
                        V.tensor_tensor(PERCb[:, zstep, :], uu[:], ppb[:], Alu.min)
                    SMb = stile("SMb")
                    V.tensor_tensor(SMb[:], SMa[:], rech[:], Alu.subtract)
                    SMc = stile("SMc")
                    V.tensor_tensor(SMc[:], SMb[:], FCb[:], Alu.min)
                    if z:
                        vv = stile("vv")
                        V.tensor_tensor(vv[:], uu[:], PERCb[:, zstep, :], Alu.subtract)
                    a1 = stile("a1")
                    V.tensor_tensor(a1[:], SMc[:], hv(Fch, u), Alu.mult)
                    a2 = stile("a2")
                    V.tensor_tensor(a2[:], SMc[:], hv(Eh, u), Alu.subtract)
                    if z:
                        t1 = stile("t1")
                        V.tensor_tensor(t1[:], vv[:], uzlb[:], Alu.min)
                    V.tensor_tensor(d2s[:], a1[:], a2[:], Alu.max)
                    V.tensor_tensor(rub[:, u, :], SMa[:], SMc[:], Alu.subtract)
                    if z:
                        x0 = stile("x0")
                        V.tensor_tensor(x0[:], vv[:], t1[:], Alu.subtract)
                    zF = stile("zF")
                    V.tensor_tensor(zF[:], SMc[:], hv(Fh, u), Alu.mult)
                    V.tensor_tensor(ETb[:, u, :], zF[:], hv(Eh, u), Alu.min)
                    if z:
                        V.tensor_tensor(Q0b[:, zstep, :], k0b[:], x0[:], Alu.mult)
                        V.tensor_tensor(yb[:, zstep, :], vv[:], Q0b[:, zstep, :],
                                        Alu.subtract)
                        V.tensor_tensor(SUZs[:], omk1b[:], yb[:, zstep, :], Alu.mult)

                for t in range(1, TC):
                    u, v = t - 1, t - 2  # soil-tail step, suz step
                    z = v >= 0
                    # 26-slot cycle; SM loop (rech->SMb->SMc->a1/a2->d2->m0->
                    # ln->w1->exp->wrap rech) saturates it with 7-slot gaps
                    # around both Act round trips.
                    # s0 rech(u) = (sw + 0)*i; the zero gate tile is
                    # written late in the previous cycle so the scheduler
                    # cannot enqueue rech before the exp result is near
                    rech = stile("rech")
                    V.tensor_tensor(rech[:], sw_prev[:], ib[:, u, :], Alu.mult)
                    # s1 SMa(u)
                    SMa = stile("SMa")
                    V.scalar_tensor_tensor(SMa[:], d2s[:], PRECS, ib[:, u, :],
                                           Alu.max, Alu.add)
                    # s2 z1: uu = SUZ + ru[v]
                    if z:
                        uu = stile("uu")
                        V.tensor_tensor(uu[:], SUZs[:], rub[:, v, :], Alu.add)
                    # s3 SMb(u)
                    SMb = stile("SMb")
                    V.tensor_tensor(SMb[:], SMa[:], rech[:], Alu.subtract)
                    # s4 z2: PERC
                    if z:
                        V.tensor_tensor(PERCb[:, v, :], uu[:], ppb[:], Alu.min)
                    # s5 SMc(u)
                    SMc = stile("SMc")
                    V.tensor_tensor(SMc[:], SMb[:], FCb[:], Alu.min)
                    # s6 z3: vv
                    if z:
                        vv = stile("vv")
                        V.tensor_tensor(vv[:], uu[:], PERCb[:, v, :], Alu.subtract)
                    # s7 a1 = SMc*Fc[u]
                    a1 = stile("a1")
                    V.tensor_tensor(a1[:], SMc[:], hv(Fch, u), Alu.mult)
                    # s8 a2 = SMc - E[u]
                    a2 = stile("a2")
                    V.tensor_tensor(a2[:], SMc[:], hv(Eh, u), Alu.subtract)
                    # s9 z4: t1
                    if z:
                        t1 = stile("t1")
                        V.tensor_tensor(t1[:], vv[:], uzlb[:], Alu.min)
                    # s10 d2 = max(a1, a2)
                    V.tensor_tensor(d2s[:], a1[:], a2[:], Alu.max)
                    # s11 z5: x0
                    if z:
                        x0 = stile("x0")
                        V.tensor_tensor(x0[:], vv[:], t1[:], Alu.subtract)
                    # s12 m0(t) -> ln
                    m0 = stile("m0")
                    V.scalar_tensor_tensor(m0[:], d2s[:], PRECS, iFCb[:],
                                           Alu.max, Alu.mult)
                    S.activation(m0[:], m0[:], Act.Ln); l = m0
                    # s13 z6: Q0
                    if z:
                        V.tensor_tensor(Q0b[:, v, :], k0b[:], x0[:], Alu.mult)
                    # s14 n2: SP' (snow stays on DVE: Pool HW lacks
                    # TensorScalarPtr and min/max TensorTensor)
                    V.scalar_tensor_tensor(SW48[:, 0:GH], UW[:, 0:GH], 0.0,
                                           UW[:, GH:48], Alu.max, Alu.min)
                    # s15 z7: y
                    if z:
                        V.tensor_tensor(yb[:, v, :], vv[:], Q0b[:, v, :], Alu.subtract)
                    # s16 n3: v = kc*SP'
                    v_ = stile("v_")
                    V.tensor_tensor(v_[:], kcb[:], SW48[:, 0:GH], Alu.mult)
                    # s17 z8: SUZ'
                    if z:
                        V.tensor_tensor(SUZs[:], omk1b[:], yb[:, v, :], Alu.mult)
                    # s18 n4: W'
                    V.tensor_tensor(SW48[:, GH:48], v_[:], UW[:, GH:48], Alu.min)
                    # s19 w1 = (l + 0)*beta -> exp (gated like rech)
                    w1 = stile("w1")
                    V.tensor_tensor(w1[:], betab[:], l[:], Alu.mult)
                    S.activation(w1[:], w1[:], Act.Exp); sw = w1
                    # s20 n5: q
                    q = stile("q")
                    V.tensor_tensor(q[:], UW[:, GH:48], SW48[:, GH:48], Alu.subtract)
                    # s21 zF(u) = SMc * F[u]   (exact ET path, off the SM loop)
                    zF = stile("zF")
                    V.tensor_tensor(zF[:], SMc[:], hv(Fh, u), Alu.mult)
                    # s22 n6: i
                    V.tensor_tensor(ib[:, t, :], hv(Rh, t), q[:], Alu.add)
                    # s23 ET(u) = min(zF, E[u])
                    V.tensor_tensor(ETb[:, u, :], zF[:], hv(Eh, u), Alu.min)
                    # s24 n1+: UW(t+1)
                    if t < TC - 1:
                        UW = stile("UW", 48)
                        V.tensor_tensor(UW[:], SW48[:], hv(ABh, t + 1), Alu.add)
                    # s25 ru(u)
                    V.tensor_tensor(rub[:, u, :], SMa[:], SMc[:], Alu.subtract)
                    sw_prev = sw

                    if t == 36:
                        # prefetch next chunk's AB/R h1 (their h1 reads end at
                        # cycle 36); overlaps the remaining cycles
                        for bufs in (ABh, Rh):
                            sl = in_shift[id(bufs)][(ci + 1) * TC:
                                                    (ci + 1) * TC + IH] \
                                .rearrange("t p g -> p t g")
                            nc.sync.dma_start(out=bufs[0][:], in_=sl)
                        # first-half post (outputs complete through OH-1)
                        emit_post(ci, 0, OH, SUBS_H1)
                    elif t == 37:
                        # Fc/E reads lag one cycle (index u=t-1): h1 clear now
                        for bufs in (Fch, Fh, Eh):
                            sl = in_shift[id(bufs)][(ci + 1) * TC:
                                                    (ci + 1) * TC + IH] \
                                .rearrange("t p g -> p t g")
                            nc.sync.dma_start(out=bufs[0][:], in_=sl)

                # epilogue: soil tail for TC-1 with suz TC-2, then suz TC-1
                tail_ops(TC - 1, sw_prev, TC - 2)
                vv = stile("vv"); uu = stile("uu")
                s = TC - 1
                V.tensor_tensor(uu[:], SUZs[:], rub[:, s, :], Alu.add)
                V.tensor_tensor(PERCb[:, s, :], uu[:], ppb[:], Alu.min)
                V.tensor_tensor(vv[:], uu[:], PERCb[:, s, :], Alu.subtract)
                t1 = stile("t1"); V.tensor_tensor(t1[:], vv[:], uzlb[:], Alu.min)
                x0 = stile("x0"); V.tensor_tensor(x0[:], vv[:], t1[:], Alu.subtract)
                V.tensor_tensor(Q0b[:, s, :], k0b[:], x0[:], Alu.mult)
                V.tensor_tensor(yb[:, s, :], vv[:], Q0b[:, s, :], Alu.subtract)
                V.tensor_tensor(SUZs[:], omk1b[:], yb[:, s, :], Alu.mult)

                # second-half post
                emit_post(ci, OH, TC - OH, SUBS_H2)

            # ---- finale: routing conv + output assembly ----
            gparts = [(0, 128), (128, 64)]
            wtaps = []
            for (p0, pl) in gparts:
                rts = post.tile([pl, 2], f32, tag=f"rts{p0}", name=f"rts{p0}")
                nc.sync.dma_start(out=rts[:], in_=rt_ap[p0:p0 + pl, :])
                am1 = post.tile([pl, 1], f32, tag=f"am1{p0}", name=f"am1{p0}")
                V.tensor_scalar(am1[:], rts[:, 0:1], 2.9, 0.0, Alu.mult, Alu.max)
                V.tensor_scalar(am1[:], am1[:], -0.9, None, Alu.add)
                th = post.tile([pl, 1], f32, tag=f"th{p0}", name=f"th{p0}")
                V.tensor_scalar(th[:], rts[:, 1:2], 6.5, 0.0, Alu.mult, Alu.max)
                V.tensor_scalar(th[:], th[:], 0.5, None, Alu.add)
                ivt = post.tile([pl, 1], f32, tag=f"ivt{p0}", name=f"ivt{p0}")
                V.reciprocal(ivt[:], th[:])
                lnt = post.tile([pl, LENF], f32, tag=f"lnt{p0}", name=f"lnt{p0}")
                nc.sync.dma_start(out=lnt[:], in_=lntk_ap.to_broadcast([pl, LENF]))
                tkb = post.tile([pl, LENF], f32, tag=f"tkb{p0}", name=f"tkb{p0}")
                nc.sync.dma_start(out=tkb[:], in_=tk_ap.to_broadcast([pl, LENF]))
                e1 = post.tile([pl, LENF], f32, tag=f"e1{p0}", name=f"e1{p0}")
                V.tensor_scalar(e1[:], lnt[:], am1[:, 0:1], None, Alu.mult)
                e2 = post.tile([pl, LENF], f32, tag=f"e2{p0}", name=f"e2{p0}")
                V.tensor_scalar(e2[:], tkb[:], ivt[:, 0:1], None, Alu.mult)
                V.tensor_tensor(e1[:], e1[:], e2[:], Alu.subtract)
                vt = post.tile([pl, LENF], f32, tag=f"vt{p0}", name=f"vt{p0}")
                S.activation(vt[:], e1[:], Act.Exp)
                ssum = post.tile([pl, 1], f32, tag=f"ss{p0}", name=f"ss{p0}")
                V.tensor_reduce(ssum[:], vt[:], mybir.AxisListType.X, Alu.add)
                rs = post.tile([pl, 1], f32, tag=f"rs{p0}", name=f"rs{p0}")
                V.reciprocal(rs[:], ssum[:])
                wt = post.tile([pl, LENF], f32, tag=f"wt{p0}", name=f"wt{p0}")
                V.tensor_scalar(wt[:], vt[:], rs[:, 0:1], None, Alu.mult)
                wtaps.append(wt)

            tblocks = [(i * 128, min(128, NSTEP - i * 128)) for i in range(6)]

            # transposed-load Qsimave into [grid, time] padded tiles
            xps = []
            for (p0, pl) in gparts:
                xp = post.tile([pl, LENF - 1 + NSTEP], f32, tag=f"xp{p0}", name=f"xp{p0}")
                V.memset(xp[:], 0.0)
                xps.append(xp)
            for (t0, tl) in tblocks:
                ld = post.tile([128, G], f32, tag="qsld", name="qsld")
                nc.sync.dma_start(out=ld[:tl, :], in_=scr["smqs"][t0:t0 + tl, :])
                for xi, (p0, pl) in enumerate(gparts):
                    pst = psum.tile([pl, 128], f32, tag="ptr", name="ptr")
                    nc.tensor.transpose(pst[:, :tl], ld[:tl, p0:p0 + pl],
                                        id_sb[:tl, :tl])
                    V.tensor_copy(out=xps[xi][:, LENF - 1 + t0:LENF - 1 + t0 + tl],
                                  in_=pst[:, :tl])

            accs = []
            for xi, (p0, pl) in enumerate(gparts):
                acc = post.tile([pl, NSTEP], f32, tag=f"acc{p0}", name=f"acc{p0}")
                acc2 = post.tile([pl, NSTEP], f32, tag=f"acc2{p0}", name=f"acc2{p0}")
                xp, wt = xps[xi], wtaps[xi]
                V.tensor_scalar(acc[:], xp[:, LENF - 1:LENF - 1 + NSTEP],
                                wt[:, 0:1], None, Alu.mult)
                cur, nxt = acc, acc2
                for k in range(1, LENF):
                    V.scalar_tensor_tensor(
                        nxt[:], xp[:, LENF - 1 - k:LENF - 1 - k + NSTEP],
                        wt[:, k:k + 1], cur[:], Alu.mult, Alu.add)
                    cur, nxt = nxt, cur
                accs.append(cur)

            for (t0, tl) in tblocks:
                packs = post.tile([128, G, 5], f32, tag="packs", name="packs")
                for j, name in enumerate(["smq0", "smq1", "smq2", "smet"]):
                    ld = post.tile([128, G], f32, tag=f"mld{j}", name=f"mld{j}")
                    nc.sync.dma_start(out=ld[:tl, :], in_=scr[name][t0:t0 + tl, :])
                    V.tensor_copy(out=packs[:tl, :, j + 1], in_=ld[:tl, :])
                for xi, (p0, pl) in enumerate(gparts):
                    pst = psum.tile([128, 128], f32, tag="ptr2", name="ptr2")
                    nc.tensor.transpose(pst[:tl, :pl], accs[xi][:, t0:t0 + tl],
                                        id_sb[:pl, :pl])
                    V.tensor_copy(out=packs[:tl, p0:p0 + pl, 0], in_=pst[:tl, :pl])
                nc.sync.dma_start(out=out_ap[t0:t0 + tl, :, :], in_=packs[:tl, :, :])

    nc.compile()
    return nc


def _prep_inputs(x, parameters, rtwts):
    x = np.ascontiguousarray(np.asarray(x, np.float32))
    parameters = np.ascontiguousarray(np.asarray(parameters, np.float32))
    rtwts = np.ascontiguousarray(np.asarray(rtwts, np.float32))
    NPAD = NC_CORES * G
    xp = np.zeros((NSTEP, NPAD, 3), np.float32)
    xp[:, :NGRID] = x
    pp = np.full((NPAD, 12, MU), 0.5, np.float32)
    pp[:NGRID] = parameters
    rp = np.full((NPAD, 2), 0.5, np.float32)
    rp[:NGRID] = rtwts

    # physical params per (grid, mu): [NPAD, 12, MU]
    lo = PARA_SCALE[:, 0][None, :, None]
    hi = PARA_SCALE[:, 1][None, :, None]
    phys = lo + pp * (hi - lo)
    (BETA, FC, K0, K1, K2, LP, PPc, UZL, TT, CFMAX, CFR, CWH) = \
        [phys[:, j, :] for j in range(12)]

    Pfull, Tfull, Efull = xp[:, :, 0], xp[:, :, 1], xp[:, :, 2]  # [NSTEP, NPAD]

    wmean = np.zeros((P, GL), np.float32)
    for p in range(P):
        wmean[p, p // MU] = 1.0 / MU
    id128 = np.eye(P, dtype=np.float32)
    tk = (np.arange(LENF, dtype=np.float32) + 0.5).reshape(1, LENF)
    lntk = np.log(tk).astype(np.float32)

    def lanes(arr_g_mu, sl):
        # [192, MU] (grid-major) -> [P=GL*MU, GH] lane layout
        a = arr_g_mu[sl].reshape(GL, GH, MU).transpose(0, 2, 1)  # gl, mu, gh
        return np.ascontiguousarray(a.reshape(P, GH).astype(np.float32))

    in_maps = []
    for c in range(NC_CORES):
        sl = slice(c * G, (c + 1) * G)
        TTl = lanes(TT, sl); CFMAXl = lanes(CFMAX, sl); CFRl = lanes(CFR, sl)
        LPl = lanes(LP, sl); FCl = lanes(FC, sl)

        # [NSTEP, 192] -> [NSTEP, GL, 1, GH] -> broadcast mu -> [NSTEP, P, GH]
        def blanes(arr):
            a = arr[:, sl].reshape(NSTEP, GL, GH)
            a = np.broadcast_to(a[:, :, None, :], (NSTEP, GL, MU, GH))
            return a.reshape(NSTEP, P, GH)

        Pl, Tl, El = blanes(Pfull), blanes(Tfull), blanes(Efull)
        db = Tl - TTl[None]
        ge = (db >= 0.0).astype(np.float32)
        R = Pl * ge
        s = Pl - R
        mcap = np.maximum(CFMAXl[None] * db, 0.0)
        ccap = np.maximum(-(CFRl * CFMAXl)[None] * db, 0.0)
        ab = s - mcap + ccap
        NPADT = NSTEP + TC
        ab48 = np.zeros((NPADT, P, 48), np.float32)
        ab48[:NSTEP, :, 0:GH] = ab
        ab48[:NSTEP, :, GH:48] = s
        F = np.minimum(El / (LPl * FCl)[None], 1.0).astype(np.float32)
        Fc = (1.0 - F).astype(np.float32)

        def padt(arr):
            out = np.zeros((NPADT, P, GH), np.float32)
            out[:NSTEP] = arr
            return out

        parc = pp[sl].reshape(GL, GH, 12, MU).transpose(0, 3, 2, 1)
        in_maps.append({
            "ab48": np.ascontiguousarray(ab48),
            "rb": np.ascontiguousarray(padt(R)),
            "fcb": np.ascontiguousarray(padt(Fc)),
            "fb": np.ascontiguousarray(padt(F)),
            "eb": np.ascontiguousarray(padt(El)),
            "par": np.ascontiguousarray(parc.reshape(P, 12, GH)),
            "rt": np.ascontiguousarray(rp[sl]),
            "wmean": wmean, "id128": id128, "lntk": lntk, "tk": tk,
        })
    return in_maps


def kernel(x, parameters, rtwts, mu, _want_trace=False):
    assert int(mu) == MU
    from concourse.bass_utils import run_bass_kernel_spmd
    if "prog" not in _PROGRAM_CACHE:
        _PROGRAM_CACHE["prog"] = _build_program()
    nc = _PROGRAM_CACHE["prog"]
    in_maps = _prep_inputs(x, parameters, rtwts)
    res = run_bass_kernel_spmd(nc, in_maps, core_ids=list(range(NC_CORES)),
                               trace=_want_trace)
    outs = [r["out"] for r in res.results]  # each [730, 192, 5]
    full = np.concatenate(outs, axis=1)[:, :NGRID, :]
    if _want_trace:
        _PROGRAM_CACHE["last_results"] = res
    return np.ascontiguousarray(full.astype(np.float32))


# revision 29
# speedup vs baseline: 1.1822x; 1.0280x over previous
"""HBV hydrology model (nn_HBVMul) Trainium2 Bass kernel — v2.

Sharding: data-parallel over the 1500-grid axis across 8 cores (192 grids/core,
padded to 1536). Per-core lane layout: partition p = g_lo*16 + mu (g_lo in 0..7,
mu in 0..15), free dim g_hi in 0..23; local grid = g_lo*24 + g_hi.

v2 restructure (validated in numpy against the jax reference and on HW):
  - Exogenous per-step series (snowfall s, melt/refreeze budget ab, rain R,
    evap factors F = min(E/(LP*FC), 1) and Fc = 1-F) are computed host-side
    and DMA'd in; they are pure elementwise transforms of the inputs.
  - Activation-table pin: Ln/Exp forced into the shared
    natural_log_exp_and_others table (else the placement pass reloads the
    act table on every Ln<->Exp switch, ~1.3us twice per step).
  - Software-pipelined step cycle (~26 ops): cycle t runs the soil tail of
    step t-1 (rech..d2 via d2 = max(SMc*Fc, SMc-E), ET = min(SMc*F, E)
    exactly off the critical loop), the soil head of step t (m0 -> Act ln ->
    w1 -> Act exp), snow for step t, and the SUZ chain for step t-2 —
    ordered so every RAW dependency has >=1 unrelated op between producer
    and consumer (dependent back-to-back DVE ops cost ~210ns vs ~122ns) and
    both Act round trips have ~7 filler slots.
  - Inputs split into halves: h1 of chunk ci+1 is DMA-prefetched mid-body
    through +TC-shifted dram views (tensors padded by TC rows); the post
    pass (Q1/Q2/Qs on Pool, folded SLZ scan z' = (PERC+z)*(1-k2) on DVE,
    mu-mean matmuls on PE, staging copies on Act) runs in two halves so it
    overlaps the step loop.
  - Pool HW ISA only supports TT add/sub/mult (+ TensorScalar): no
    TensorScalarPtr/min/max there, so the state chains all stay on DVE.
"""

import os
import sys
import numpy as np

sys.path.insert(0, "/opt/trn_rl_repo")

NSTEP, NGRID, MU, LENF = 730, 1500, 16, 15
PRECS = 1e-5
NC_CORES = 8
G = 192          # grids per core
GL, GH = 8, 24   # g_lo x g_hi split of the 192 grids
P = 128          # partitions = GL * MU
NCH, TC = 10, 73  # time chunks
TSUB = [(0, 19), (19, 18), (37, 18), (55, 18)]  # mu-mean matmul sub-slices

PARA_SCALE = np.array([[1, 6], [50, 1000], [0.05, 0.9], [0.01, 0.5], [0.001, 0.2],
                       [0.2, 1], [0, 10], [0, 100], [-2.5, 2.5], [0.5, 10],
                       [0, 0.1], [0, 0.2]], dtype=np.float32)
ROUT_SCALE = np.array([[0, 2.9], [0, 6.5]], dtype=np.float32)

_PROGRAM_CACHE = {}


def _build_program():
    import concourse.bacc as bacc
    import concourse.bass as bass
    import concourse.tile as tile
    import concourse.mybir as mybir
    from concourse.bass import ts

    dt = mybir.dt
    Alu = mybir.AluOpType
    Act = mybir.ActivationFunctionType

    # Pin Ln/Exp to the combined activation table (json index 6,
    # natural_log_exp_and_others). The table-load placement pass picks, per
    # activation, some table containing its function; with Ln in
    # natural_log (5) and Exp in exp_and_others (0) it reloads the table on
    # every Ln<->Exp switch (~1.3us each, twice per step). Hiding Ln/Exp
    # from all other tables (ids/order unchanged, so the emitted
    # act_func_set_id stays valid) forces the shared table and hoists the
    # single load out of the loop.
    from concourse.hw_specs import get_activation_tables as _gat

    def _gat_pinned(arch):
        tabs = _gat(arch)
        keep = "natural_log_exp_and_others"
        drop = {mybir.ActivationFunctionType.Ln, mybir.ActivationFunctionType.Exp}
        return {name: (funcs if name == keep else funcs - drop)
                for name, funcs in tabs.items()}

    bacc.get_activation_tables = _gat_pinned

    nc = bacc.Bacc("TRN2", target_bir_lowering=False, debug=False,
                   num_devices=NC_CORES)

    f32 = dt.float32
    # padded by TC rows so the mid-chunk prefetch of chunk ci+1 stays in range
    NPADT = NSTEP + TC
    ab_ap = nc.dram_tensor("ab48", [NPADT, P, 48], f32, kind="ExternalInput").ap()
    rb_ap = nc.dram_tensor("rb", [NPADT, P, GH], f32, kind="ExternalInput").ap()
    fcb_ap = nc.dram_tensor("fcb", [NPADT, P, GH], f32, kind="ExternalInput").ap()
    fb_ap = nc.dram_tensor("fb", [NPADT, P, GH], f32, kind="ExternalInput").ap()
    eb_ap = nc.dram_tensor("eb", [NPADT, P, GH], f32, kind="ExternalInput").ap()
    par_ap = nc.dram_tensor("par", [P, 12, GH], f32, kind="ExternalInput").ap()
    rt_ap = nc.dram_tensor("rt", [G, 2], f32, kind="ExternalInput").ap()
    wm_ap = nc.dram_tensor("wmean", [P, GL], f32, kind="ExternalInput").ap()
    id_ap = nc.dram_tensor("id128", [P, P], f32, kind="ExternalInput").ap()
    lntk_ap = nc.dram_tensor("lntk", [1, LENF], f32, kind="ExternalInput").ap()
    tk_ap = nc.dram_tensor("tk", [1, LENF], f32, kind="ExternalInput").ap()
    out_ap = nc.dram_tensor("out", [NSTEP, G, 5], f32, kind="ExternalOutput").ap()

    scr = {}
    for name in ["smqs", "smq0", "smq1", "smq2", "smet"]:
        scr[name] = nc.dram_tensor(name, [NSTEP, G], f32, kind="Internal").ap()

    with tile.TileContext(nc) as tc:
        from contextlib import ExitStack
        ctx = ExitStack()
        with ctx:
            consts = ctx.enter_context(tc.tile_pool(name="consts", bufs=1))
            chunk = ctx.enter_context(tc.tile_pool(name="chunk", bufs=1))
            step = ctx.enter_context(tc.tile_pool(name="step", bufs=24))
            post = ctx.enter_context(tc.tile_pool(name="post", bufs=2))
            psum = ctx.enter_context(tc.tile_pool(name="psum", bufs=2, space="PSUM"))

            V = nc.vector
            S = nc.scalar
            GP = nc.gpsimd

            # ---- Phase 0: parameters ----
            par_sb = consts.tile([P, 12, GH], f32)
            nc.sync.dma_start(out=par_sb[:], in_=par_ap)
            pp_ = {}
            # param indices: 0 BETA, 1 FC, 2 K0, 3 K1, 4 K2, 5 LP, 6 PERC,
            #                7 UZL, 8 TT, 9 CFMAX, 10 CFR, 11 CWH
            for j in (0, 1, 2, 3, 4, 6, 7, 11):
                pt = consts.tile([P, GH], f32, tag=f"par{j}", name=f"par{j}")
                lo, hi = float(PARA_SCALE[j, 0]), float(PARA_SCALE[j, 1])
                V.tensor_scalar(pt[:], par_sb[:, j, :], hi - lo, lo,
                                Alu.mult, Alu.add)
                pp_[j] = pt
            betab, FCb, k0b, k1b, k2b = pp_[0], pp_[1], pp_[2], pp_[3], pp_[4]
            ppb, uzlb, CWHb = pp_[6], pp_[7], pp_[11]

            def ctile(tag, w=GH):
                return consts.tile([P, w], f32, tag=tag, name=tag)

            kcb = ctile("kcb");     V.tensor_scalar(kcb[:], CWHb[:], 1.0, None, Alu.add)
            iFCb = ctile("ifc");    V.reciprocal(iFCb[:], FCb[:])
            omk1b = ctile("omk1");  V.tensor_scalar(omk1b[:], k1b[:], -1.0, 1.0, Alu.mult, Alu.add)
            aslzb = ctile("aslz");  V.tensor_scalar(aslzb[:], k2b[:], -1.0, 1.0, Alu.mult, Alu.add)
            ralz = ctile("ralz");   V.reciprocal(ralz[:], aslzb[:])
            kqb = ctile("kqb");     V.tensor_tensor(kqb[:], k2b[:], ralz[:], Alu.mult)

            wm_sb = consts.tile([P, GL], f32)
            nc.sync.dma_start(out=wm_sb[:], in_=wm_ap)
            id_sb = consts.tile([P, P], f32)
            nc.sync.dma_start(out=id_sb[:], in_=id_ap)

            # ---- states ----
            SW48 = consts.tile([P, 48], f32, tag="SW48", name="SW48")
            V.memset(SW48[:, 0:GH], 0.001)   # SP
            V.memset(SW48[:, GH:48], 0.002)  # W = SNOWPACK + MELTWATER
            d2s = consts.tile([P, GH], f32, tag="d2s", name="d2s")
            V.memset(d2s[:], 0.001)          # raw SM (clamp applied at use)
            SUZs = consts.tile([P, GH], f32, tag="SUZ", name="SUZ")
            V.memset(SUZs[:], 0.001)
            SLZl = consts.tile([P, GH], f32, tag="SLZ", name="SLZ")
            V.memset(SLZl[:], 0.001)

            # ---- chunk buffers ----
            IH = 37   # input half split: h1=[0,IH) prefetched, h2=[IH,TC)
            OH = 34   # output half split for the mid-chunk post pass

            def cbuf(tag, n, w=GH):
                return chunk.tile([P, n, w], f32, tag=tag, name=tag)
            ABh = [cbuf("ABh1", IH, 48), cbuf("ABh2", TC - IH, 48)]
            Rh = [cbuf("Rh1", IH), cbuf("Rh2", TC - IH)]
            Fch = [cbuf("Fch1", IH), cbuf("Fch2", TC - IH)]
            Fh = [cbuf("Fh1", IH), cbuf("Fh2", TC - IH)]
            Eh = [cbuf("Eh1", IH), cbuf("Eh2", TC - IH)]

            def hv(bufs, t):  # half-split view of input series at step t
                return bufs[0][:, t, :] if t < IH else bufs[1][:, t - IH, :]

            ib = cbuf("ib", TC); rub = cbuf("rub", TC)
            PERCb = cbuf("PERCb", TC); Q0b = cbuf("Q0b", TC); yb = cbuf("yb", TC)
            ETb = cbuf("ETb", TC); Q1b = cbuf("Q1b", TC)
            zb = cbuf("zb", TC)
            Q2b = cbuf("Q2b", TC); Qsb = cbuf("Qsb", TC)

            def bcr(t, n):  # broadcast [P, GH] param over n steps
                return t[:, None, :].to_broadcast([P, n, GH])

            scr_views = {k: v.rearrange("(c t) (gl gh) -> c gl t gh", c=NCH, gl=GL)
                         for k, v in scr.items()}

            def stile(tag, w=GH):
                return step.tile([P, w], f32, tag=tag, name=tag)

            def emit_post(ci, h0, hn, subs):
                """Batched post for output positions [h0, h0+hn): Q1/q2t/Q2/Qs
                on Pool, SLZ scans on DVE, mu-means on PE."""
                sl = slice(h0, h0 + hn)
                GP.tensor_tensor(Q1b[:, sl, :], bcr(k1b, hn), yb[:, sl, :], Alu.mult)
                for g in range(GH):
                    # z' = (PERC[t] + z) * (1-k2): SLZ update as one scan
                    V.tensor_tensor_scan(
                        zb[:, sl, g], PERCb[:, sl, g],
                        aslzb[:, g:g + 1].to_broadcast([P, hn]),
                        SLZl[:, g:g + 1], Alu.add, Alu.mult)
                V.tensor_copy(out=SLZl[:], in_=zb[:, h0 + hn - 1, :])
                GP.tensor_tensor(Q2b[:, sl, :], zb[:, sl, :], bcr(kqb, hn), Alu.mult)
                GP.tensor_tensor(Qsb[:, sl, :], Q0b[:, sl, :], Q1b[:, sl, :], Alu.add)
                GP.tensor_tensor(Qsb[:, sl, :], Qsb[:, sl, :], Q2b[:, sl, :], Alu.add)
                for buf, name in ((Qsb, "smqs"), (Q0b, "smq0"), (Q1b, "smq1"),
                                  (Q2b, "smq2"), (ETb, "smet")):
                    for (t0, tl) in subs:
                        ps = psum.tile([GL, 20 * GH], f32, tag="msum", name="msum")
                        rhs = buf[:, t0:t0 + tl, :].rearrange("p t g -> p (t g)")
                        nc.tensor.matmul(ps[:, :tl * GH], wm_sb[:], rhs,
                                         start=True, stop=True)
                        stg = post.tile([GL, 20 * GH], f32, tag="mstg", name="mstg")
                        S.copy(stg[:, :tl * GH], ps[:, :tl * GH])
                        dst = scr_views[name][ci][:, t0:t0 + tl, :]
                        nc.sync.dma_start(
                            out=dst,
                            in_=stg[:, :tl * GH].rearrange("m (t g) -> m t g", g=GH))

            SUBS_H1 = [(0, 17), (17, 17)]
            SUBS_H2 = [(OH, 20), (OH + 20, TC - OH - 20)]
            in_srcs = ((ABh, ab_ap), (Rh, rb_ap), (Fch, fcb_ap),
                       (Fh, fb_ap), (Eh, eb_ap))
            # base views for the mid-chunk prefetch of chunk ci+1
            in_shift = {id(bufs): srcap for bufs, srcap in in_srcs}

            # preload chunk 0 h1 before the loop
            for bufs, srcap in in_srcs:
                sl = srcap[0:IH].rearrange("t p g -> p t g")
                nc.sync.dma_start(out=bufs[0][:], in_=sl)

            # Software-pipelined step cycle. Cycle t computes, interleaved so
            # every RAW dependency has >=1 unrelated op between producer and
            # consumer (dependent back-to-back DVE ops cost ~210ns vs ~122ns):
            #   soil tail of step t-1 (rech..d2/ET/ru, using sw issued last
            #   cycle, so the Act exp round-trip has a full cycle of slack),
            #   soil head of step t (m0 -> ln -> w1 -> exp),
            #   snow for step t, and the full SUZ chain for step t-2.
            for ci in range(NCH):
                # current chunk h2 loads (h1 prefetched during the previous
                # iteration; chunk 0 h1 preloaded above)
                for bufs, srcap in in_srcs:
                    sl = srcap[ci * TC + IH:(ci + 1) * TC]                         .rearrange("t p g -> p t g")
                    nc.sync.dma_start(out=bufs[1][:], in_=sl)

                def head_ops(t, UW):
                    """soil head + snow for step t; returns (UW_next, sw)."""
                    # emitted inline by emit_cycle; only used for t=0 prologue
                    m0 = stile("m0")
                    V.scalar_tensor_tensor(m0[:], d2s[:], PRECS, iFCb[:],
                                           Alu.max, Alu.mult)
                    S.activation(m0[:], m0[:], Act.Ln); l = m0
                    V.scalar_tensor_tensor(SW48[:, 0:GH], UW[:, 0:GH], 0.0,
                                           UW[:, GH:48], Alu.max, Alu.min)
                    v_ = stile("v_")
                    V.tensor_tensor(v_[:], kcb[:], SW48[:, 0:GH], Alu.mult)
                    V.tensor_tensor(SW48[:, GH:48], v_[:], UW[:, GH:48], Alu.min)
                    w1 = stile("w1"); V.tensor_tensor(w1[:], betab[:], l[:], Alu.mult)
                    S.activation(w1[:], w1[:], Act.Exp); sw = w1
                    q = stile("q")
                    V.tensor_tensor(q[:], UW[:, GH:48], SW48[:, GH:48], Alu.subtract)
                    V.tensor_tensor(ib[:, t, :], hv(Rh, t), q[:], Alu.add)
                    UW2 = stile("UW", 48)
                    V.tensor_tensor(UW2[:], SW48[:], hv(ABh, t + 1), Alu.add)
                    return UW2, sw

                # t=0 prologue
                UW = stile("UW", 48)
                V.tensor_tensor(UW[:], SW48[:], hv(ABh, 0), Alu.add)
                UW, sw_prev = head_ops(0, UW)

                def tail_ops(u, sw_u, zstep):
                    """soil tail for step u interleaved with the SUZ chain for
                    step zstep (None to skip); emits in dependency-spaced
                    order. Returns nothing; updates d2s/SUZs states."""
                    z = zstep is not None
                    rech = stile("rech")
                    V.tensor_tensor(rech[:], sw_u[:], ib[:, u, :], Alu.mult)
                    if z:
                        uu = stile("uu")
                        V.tensor_tensor(uu[:], SUZs[:], rub[:, zstep, :], Alu.add)
                    SMa = stile("SMa")
                    V.scalar_tensor_tensor(SMa[:], d2s[:], PRECS, ib[:, u, :],
                                           Alu.max, Alu.add)
                    if z:
                        V.tensor_tensor(PERCb[:, zstep, :], uu[:], ppb[:], Alu.min)
                    SMb = stile("SMb")
                    V.tensor_tensor(SMb[:], SMa[:], rech[:], Alu.subtract)
                    SMc = stile("SMc")
                    V.tensor_tensor(SMc[:], SMb[:], FCb[:], Alu.min)
                    if z:
                        vv = stile("vv")
                        V.tensor_tensor(vv[:], uu[:], PERCb[:, zstep, :], Alu.subtract)
                    a1 = stile("a1")
                    V.tensor_tensor(a1[:], SMc[:], hv(Fch, u), Alu.mult)
                    a2 = stile("a2")
                    V.tensor_tensor(a2[:], SMc[:], hv(Eh, u), Alu.subtract)
                    if z:
                        t1 = stile("t1")
                        V.tensor_tensor(t1[:], vv[:], uzlb[:], Alu.min)
                    V.tensor_tensor(d2s[:], a1[:], a2[:], Alu.max)
                    V.tensor_tensor(rub[:, u, :], SMa[:], SMc[:], Alu.subtract)
                    if z:
                        x0 = stile("x0")
                        V.tensor_tensor(x0[:], vv[:], t1[:], Alu.subtract)
                    zF = stile("zF")
                    V.tensor_tensor(zF[:], SMc[:], hv(Fh, u), Alu.mult)
                    V.tensor_tensor(ETb[:, u, :], zF[:], hv(Eh, u), Alu.min)
                    if z:
                        V.tensor_tensor(Q0b[:, zstep, :], k0b[:], x0[:], Alu.mult)
                        V.tensor_tensor(yb[:, zstep, :], vv[:], Q0b[:, zstep, :],
                                        Alu.subtract)
                        V.tensor_tensor(SUZs[:], omk1b[:], yb[:, zstep, :], Alu.mult)

                for t in range(1, TC):
                    u, v = t - 1, t - 2  # soil-tail step, suz step
                    z = v >= 0
                    # 26-slot cycle; SM loop (rech->SMb->SMc->a1/a2->d2->m0->
                    # ln->w1->exp->wrap rech) saturates it with 7-slot gaps
                    # around both Act round trips.
                    # s0 rech(u) = (sw + 0)*i; the zero gate tile is
                    # written late in the previous cycle so the scheduler
                    # cannot enqueue rech before the exp result is near
                    rech = stile("rech")
                    V.tensor_tensor(rech[:], sw_prev[:], ib[:, u, :], Alu.mult)
                    # s1 SMa(u)
                    SMa = stile("SMa")
                    V.scalar_tensor_tensor(SMa[:], d2s[:], PRECS, ib[:, u, :],
                                           Alu.max, Alu.add)
                    # s2 z1: uu = SUZ + ru[v]
                    if z:
                        uu = stile("uu")
                        V.tensor_tensor(uu[:], SUZs[:], rub[:, v, :], Alu.add)
                    # s3 SMb(u)
                    SMb = stile("SMb")
                    V.tensor_tensor(SMb[:], SMa[:], rech[:], Alu.subtract)
                    # s4 z2: PERC
                    if z:
                        V.tensor_tensor(PERCb[:, v, :], uu[:], ppb[:], Alu.min)
                    # s5 SMc(u)
                    SMc = stile("SMc")
                    V.tensor_tensor(SMc[:], SMb[:], FCb[:], Alu.min)
                    # s6 z3: vv
                    if z:
                        vv = stile("vv")
                        V.tensor_tensor(vv[:], uu[:], PERCb[:, v, :], Alu.subtract)
                    # s7 a1 = SMc*Fc[u]
                    a1 = stile("a1")
                    V.tensor_tensor(a1[:], SMc[:], hv(Fch, u), Alu.mult)
                    # s8 a2 = SMc - E[u]
                    a2 = stile("a2")
                    V.tensor_tensor(a2[:], SMc[:], hv(Eh, u), Alu.subtract)
                    # s9 z4: t1
                    if z:
                        t1 = stile("t1")
                        V.tensor_tensor(t1[:], vv[:], uzlb[:], Alu.min)
                    # s10 d2 = max(a1, a2)
                    V.tensor_tensor(d2s[:], a1[:], a2[:], Alu.max)
                    # s11 z5: x0
                    if z:
                        x0 = stile("x0")
                        V.tensor_tensor(x0[:], vv[:], t1[:], Alu.subtract)
                    # s12 m0(t) -> ln
                    m0 = stile("m0")
                    V.scalar_tensor_tensor(m0[:], d2s[:], PRECS, iFCb[:],
                                           Alu.max, Alu.mult)
                    S.activation(m0[:], m0[:], Act.Ln); l = m0
                    # s13 z6: Q0
                    if z:
                        V.tensor_tensor(Q0b[:, v, :], k0b[:], x0[:], Alu.mult)
                    # s14 n2: SP' (snow stays on DVE: Pool HW lacks
                    # TensorScalarPtr and min/max TensorTensor)
                    V.scalar_tensor_tensor(SW48[:, 0:GH], UW[:, 0:GH], 0.0,
                                           UW[:, GH:48], Alu.max, Alu.min)
                    # s15 z7: y
                    if z:
                        V.tensor_tensor(yb[:, v, :], vv[:], Q0b[:, v, :], Alu.subtract)
                    # s16 n3: v = kc*SP'
                    v_ = stile("v_")
                    V.tensor_tensor(v_[:], kcb[:], SW48[:, 0:GH], Alu.mult)
                    # s17 z8: SUZ'
                    if z:
                        V.tensor_tensor(SUZs[:], omk1b[:], yb[:, v, :], Alu.mult)
                    # s18 n4: W'
                    V.tensor_tensor(SW48[:, GH:48], v_[:], UW[:, GH:48], Alu.min)
                    # s19 w1 = (l + 0)*beta -> exp (gated like rech)
                    w1 = stile("w1")
                    V.tensor_tensor(w1[:], betab[:], l[:], Alu.mult)
                    S.activation(w1[:], w1[:], Act.Exp); sw = w1
                    # s20 n5: q
                    q = stile("q")
                    V.tensor_tensor(q[:], UW[:, GH:48], SW48[:, GH:48], Alu.subtract)
                    # s21 zF(u) = SMc * F[u]   (exact ET path, off the SM loop)
                    zF = stile("zF")
                    V.tensor_tensor(zF[:], SMc[:], hv(Fh, u), Alu.mult)
                    # s22 n6: i
                    V.tensor_tensor(ib[:, t, :], hv(Rh, t), q[:], Alu.add)
                    # s23 ET(u) = min(zF, E[u])
                    V.tensor_tensor(ETb[:, u, :], zF[:], hv(Eh, u), Alu.min)
                    # s24 n1+: UW(t+1)
                    if t < TC - 1:
                        UW = stile("UW", 48)
                        V.tensor_tensor(UW[:], SW48[:], hv(ABh, t + 1), Alu.add)
                    # s25 ru(u)
                    V.tensor_tensor(rub[:, u, :], SMa[:], SMc[:], Alu.subtract)
                    sw_prev = sw

                    if t == 36:
                        # prefetch next chunk's AB/R h1 (their h1 reads end at
                        # cycle 36); overlaps the remaining cycles
                        for bufs in (ABh, Rh):
                            sl = in_shift[id(bufs)][(ci + 1) * TC:
                                                    (ci + 1) * TC + IH] \
                                .rearrange("t p g -> p t g")
                            nc.sync.dma_start(out=bufs[0][:], in_=sl)
                        # first-half post (outputs complete through OH-1)
                        emit_post(ci, 0, OH, SUBS_H1)
                    elif t == 37:
                        # Fc/E reads lag one cycle (index u=t-1): h1 clear now
                        for bufs in (Fch, Fh, Eh):
                            sl = in_shift[id(bufs)][(ci + 1) * TC:
                                                    (ci + 1) * TC + IH] \
                                .rearrange("t p g -> p t g")
                            nc.sync.dma_start(out=bufs[0][:], in_=sl)

                # epilogue: soil tail for TC-1 with suz TC-2, then suz TC-1
                tail_ops(TC - 1, sw_prev, TC - 2)
                vv = stile("vv"); uu = stile("uu")
                s = TC - 1
                V.tensor_tensor(uu[:], SUZs[:], rub[:, s, :], Alu.add)
                V.tensor_tensor(PERCb[:, s, :], uu[:], ppb[:], Alu.min)
                V.tensor_tensor(vv[:], uu[:], PERCb[:, s, :], Alu.subtract)
                t1 = stile("t1"); V.tensor_tensor(t1[:], vv[:], uzlb[:], Alu.min)
                x0 = stile("x0"); V.tensor_tensor(x0[:], vv[:], t1[:], Alu.subtract)
                V.tensor_tensor(Q0b[:, s, :], k0b[:], x0[:], Alu.mult)
                V.tensor_tensor(yb[:, s, :], vv[:], Q0b[:, s, :], Alu.subtract)
                V.tensor_tensor(SUZs[:], omk1b[:], yb[:, s, :], Alu.mult)

                # second-half post
                emit_post(ci, OH, TC - OH, SUBS_H2)

            # ---- finale: routing conv + output assembly ----
            gparts = [(0, 128), (128, 64)]
            wtaps = []
            for (p0, pl) in gparts:
                rts = post.tile([pl, 2], f32, tag=f"rts{p0}", name=f"rts{p0}")
                nc.sync.dma_start(out=rts[:], in_=rt_ap[p0:p0 + pl, :])
                am1 = post.tile([pl, 1], f32, tag=f"am1{p0}", name=f"am1{p0}")
                V.tensor_scalar(am1[:], rts[:, 0:1], 2.9, 0.0, Alu.mult, Alu.max)
                V.tensor_scalar(am1[:], am1[:], -0.9, None, Alu.add)
                th = post.tile([pl, 1], f32, tag=f"th{p0}", name=f"th{p0}")
                V.tensor_scalar(th[:], rts[:, 1:2], 6.5, 0.0, Alu.mult, Alu.max)
                V.tensor_scalar(th[:], th[:], 0.5, None, Alu.add)
                ivt = post.tile([pl, 1], f32, tag=f"ivt{p0}", name=f"ivt{p0}")
                V.reciprocal(ivt[:], th[:])
                lnt = post.tile([pl, LENF], f32, tag=f"lnt{p0}", name=f"lnt{p0}")
                nc.sync.dma_start(out=lnt[:], in_=lntk_ap.to_broadcast([pl, LENF]))
                tkb = post.tile([pl, LENF], f32, tag=f"tkb{p0}", name=f"tkb{p0}")
                nc.sync.dma_start(out=tkb[:], in_=tk_ap.to_broadcast([pl, LENF]))
                e1 = post.tile([pl, LENF], f32, tag=f"e1{p0}", name=f"e1{p0}")
                V.tensor_scalar(e1[:], lnt[:], am1[:, 0:1], None, Alu.mult)
                e2 = post.tile([pl, LENF], f32, tag=f"e2{p0}", name=f"e2{p0}")
                V.tensor_scalar(e2[:], tkb[:], ivt[:, 0:1], None, Alu.mult)
                V.tensor_tensor(e1[:], e1[:], e2[:], Alu.subtract)
                vt = post.tile([pl, LENF], f32, tag=f"vt{p0}", name=f"vt{p0}")
                S.activation(vt[:], e1[:], Act.Exp)
                ssum = post.tile([pl, 1], f32, tag=f"ss{p0}", name=f"ss{p0}")
                V.tensor_reduce(ssum[:], vt[:], mybir.AxisListType.X, Alu.add)
                rs = post.tile([pl, 1], f32, tag=f"rs{p0}", name=f"rs{p0}")
                V.reciprocal(rs[:], ssum[:])
                wt = post.tile([pl, LENF], f32, tag=f"wt{p0}", name=f"wt{p0}")
                V.tensor_scalar(wt[:], vt[:], rs[:, 0:1], None, Alu.mult)
                wtaps.append(wt)

            tblocks = [(i * 128, min(128, NSTEP - i * 128)) for i in range(6)]

            # transposed-load Qsimave into [grid, time] padded tiles
            xps = []
            for (p0, pl) in gparts:
                xp = post.tile([pl, LENF - 1 + NSTEP], f32, tag=f"xp{p0}", name=f"xp{p0}")
                V.memset(xp[:], 0.0)
                xps.append(xp)
            for (t0, tl) in tblocks:
                ld = post.tile([128, G], f32, tag="qsld", name="qsld")
                nc.sync.dma_start(out=ld[:tl, :], in_=scr["smqs"][t0:t0 + tl, :])
                for xi, (p0, pl) in enumerate(gparts):
                    pst = psum.tile([pl, 128], f32, tag="ptr", name="ptr")
                    nc.tensor.transpose(pst[:, :tl], ld[:tl, p0:p0 + pl],
                                        id_sb[:tl, :tl])
                    V.tensor_copy(out=xps[xi][:, LENF - 1 + t0:LENF - 1 + t0 + tl],
                                  in_=pst[:, :tl])

            accs = []
            for xi, (p0, pl) in enumerate(gparts):
                acc = post.tile([pl, NSTEP], f32, tag=f"acc{p0}", name=f"acc{p0}")
                acc2 = post.tile([pl, NSTEP], f32, tag=f"acc2{p0}", name=f"acc2{p0}")
                xp, wt = xps[xi], wtaps[xi]
                V.tensor_scalar(acc[:], xp[:, LENF - 1:LENF - 1 + NSTEP],
                                wt[:, 0:1], None, Alu.mult)
                cur, nxt = acc, acc2
                for k in range(1, LENF):
                    V.scalar_tensor_tensor(
                        nxt[:], xp[:, LENF - 1 - k:LENF - 1 - k + NSTEP],
                        wt[:, k:k + 1], cur[:], Alu.mult, Alu.add)
                    cur, nxt = nxt, cur
                accs.append(cur)

            for (t0, tl) in tblocks:
                packs = post.tile([128, G, 5], f32, tag="packs", name="packs")
                for j, name in enumerate(["smq0", "smq1", "smq2", "smet"]):
                    ld = post.tile([128, G], f32, tag=f"mld{j}", name=f"mld{j}")
                    nc.sync.dma_start(out=ld[:tl, :], in_=scr[name][t0:t0 + tl, :])
                    V.tensor_copy(out=packs[:tl, :, j + 1], in_=ld[:tl, :])
                for xi, (p0, pl) in enumerate(gparts):
                    pst = psum.tile([128, 128], f32, tag="ptr2", name="ptr2")
                    nc.tensor.transpose(pst[:tl, :pl], accs[xi][:, t0:t0 + tl],
                                        id_sb[:pl, :pl])
                    V.tensor_copy(out=packs[:tl, p0:p0 + pl, 0], in_=pst[:tl, :pl])
                nc.sync.dma_start(out=out_ap[t0:t0 + tl, :, :], in_=packs[:tl, :, :])

    nc.compile()
    return nc


def _prep_inputs(x, parameters, rtwts):
    x = np.ascontiguousarray(np.asarray(x, np.float32))
    parameters = np.ascontiguousarray(np.asarray(parameters, np.float32))
    rtwts = np.ascontiguousarray(np.asarray(rtwts, np.float32))
    NPAD = NC_CORES * G
    xp = np.zeros((NSTEP, NPAD, 3), np.float32)
    xp[:, :NGRID] = x
    pp = np.full((NPAD, 12, MU), 0.5, np.float32)
    pp[:NGRID] = parameters
    rp = np.full((NPAD, 2), 0.5, np.float32)
    rp[:NGRID] = rtwts

    # physical params per (grid, mu): [NPAD, 12, MU]
    lo = PARA_SCALE[:, 0][None, :, None]
    hi = PARA_SCALE[:, 1][None, :, None]
    phys = lo + pp * (hi - lo)
    (BETA, FC, K0, K1, K2, LP, PPc, UZL, TT, CFMAX, CFR, CWH) = \
        [phys[:, j, :] for j in range(12)]

    Pfull, Tfull, Efull = xp[:, :, 0], xp[:, :, 1], xp[:, :, 2]  # [NSTEP, NPAD]

    wmean = np.zeros((P, GL), np.float32)
    for p in range(P):
        wmean[p, p // MU] = 1.0 / MU
    id128 = np.eye(P, dtype=np.float32)
    tk = (np.arange(LENF, dtype=np.float32) + 0.5).reshape(1, LENF)
    lntk = np.log(tk).astype(np.float32)

    def lanes(arr_g_mu, sl):
        # [192, MU] (grid-major) -> [P=GL*MU, GH] lane layout
        a = arr_g_mu[sl].reshape(GL, GH, MU).transpose(0, 2, 1)  # gl, mu, gh
        return np.ascontiguousarray(a.reshape(P, GH).astype(np.float32))

    in_maps = []
    for c in range(NC_CORES):
        sl = slice(c * G, (c + 1) * G)
        TTl = lanes(TT, sl); CFMAXl = lanes(CFMAX, sl); CFRl = lanes(CFR, sl)
        LPl = lanes(LP, sl); FCl = lanes(FC, sl)

        # [NSTEP, 192] -> [NSTEP, GL, 1, GH] -> broadcast mu -> [NSTEP, P, GH]
        def blanes(arr):
            a = arr[:, sl].reshape(NSTEP, GL, GH)
            a = np.broadcast_to(a[:, :, None, :], (NSTEP, GL, MU, GH))
            return a.reshape(NSTEP, P, GH)

        Pl, Tl, El = blanes(Pfull), blanes(Tfull), blanes(Efull)
        db = Tl - TTl[None]
        ge = (db >= 0.0).astype(np.float32)
        R = Pl * ge
        s = Pl - R
        mcap = np.maximum(CFMAXl[None] * db, 0.0)
        ccap = np.maximum(-(CFRl * CFMAXl)[None] * db, 0.0)
        ab = s - mcap + ccap
        NPADT = NSTEP + TC
        ab48 = np.zeros((NPADT, P, 48), np.float32)
        ab48[:NSTEP, :, 0:GH] = ab
        ab48[:NSTEP, :, GH:48] = s
        F = np.minimum(El / (LPl * FCl)[None], 1.0).astype(np.float32)
        Fc = (1.0 - F).astype(np.float32)

        def padt(arr):
            out = np.zeros((NPADT, P, GH), np.float32)
            out[:NSTEP] = arr
            return out

        parc = pp[sl].reshape(GL, GH, 12, MU).transpose(0, 3, 2, 1)
        in_maps.append({
            "ab48": np.ascontiguousarray(ab48),
            "rb": np.ascontiguousarray(padt(R)),
            "fcb": np.ascontiguousarray(padt(Fc)),
            "fb": np.ascontiguousarray(padt(F)),
            "eb": np.ascontiguousarray(padt(El)),
            "par": np.ascontiguousarray(parc.reshape(P, 12, GH)),
            "rt": np.ascontiguousarray(rp[sl]),
            "wmean": wmean, "id128": id128, "lntk": lntk, "tk": tk,
        })
    return in_maps


def kernel(x, parameters, rtwts, mu, _want_trace=False):
    assert int(mu) == MU
    from concourse.bass_utils import run_bass_kernel_spmd
    if "prog" not in _PROGRAM_CACHE:
        _PROGRAM_CACHE["prog"] = _build_program()
    nc = _PROGRAM_CACHE["prog"]
    in_maps = _prep_inputs(x, parameters, rtwts)
    res = run_bass_kernel_spmd(nc, in_maps, core_ids=list(range(NC_CORES)),
                               trace=_want_trace)
    outs = [r["out"] for r in res.results]  # each [730, 192, 5]
    full = np.concatenate(outs, axis=1)[:, :NGRID, :]
    if _want_trace:
        _PROGRAM_CACHE["last_results"] = res
    return np.ascontiguousarray(full.astype(np.float32))
